# revision 15
# baseline (speedup 1.0000x reference)
"""GCN critic network kernel for 8 TRN2 NeuronCores.

Strategy (dst-shard, no message-table collective):
  The GCNConv linear commutes with the segment sum:
      out[d] = dinv_d * (sum_{s in N(d)} dinv_s * x[s]) @ Wg.T + bg
  so each core gathers pre-scaled raw rows xd = dinv*x (host-prepped bf16
  table in DRAM, a kernel input) for its own dst shard's edges and applies
  Wg once per 128-dst block after aggregation. This removes the y-table
  AllGather of the previous design entirely; the only collective left is
  the tiny [1,128] AllReduce of the pooled vector.

  - dst nodes sharded 6250/core (padded 6272 = 49 blocks of 128).
  - Edge messages: dma_gather of 256B bf16 rows (int16 indices, node table
    split in two <=32768-row groups), segment-summed per dst block via
    one-hot matmul accumulation in PSUM:  psST[fin, slot] += seg.T @ oh.
  - One-hot built per chunk with a single DVE tensor_scalar is_equal
    (iota row constant vs per-partition dst-slot scalar) - hits the
    packed-2-byte DVE fast path.
  - Self-loops folded in as one extra matmul per block from the resident
    xdT slice (no gathered self rows).
  - h = relu(dinv * (aggT.T @ WgT)); v = ones@h blocks + colsum(x_own);
    AllReduce v; tiny MLP head with host-pretransposed weights.
"""

import os
import numpy as np
import ml_dtypes

BF16 = ml_dtypes.bfloat16
N = 50000
E = 800000
D = 128
NCORES = 8
NPC = 6250          # dst nodes per core
NPAD = 6272         # padded (49 * 128)
NB = NPAD // 128    # dst blocks per core
GRP = 32768         # int16 index-group boundary (table-row space)
SEGC = int(os.environ.get("KB_SEGC", "8"))   # chunks per gather call
DDS = int(os.environ.get("KB_DDS", "65536"))
PADSLOT = 300.0     # dst-slot sentinel for padding rows (one-hot miss)

DEBUG_BLOCKS = (int(os.environ["KB_DEBUG_BLOCKS"])
                if "KB_DEBUG_BLOCKS" in os.environ else None)
SKIP_MLP = bool(os.environ.get("KB_SKIP_MLP"))
SKIP_MAIN = bool(os.environ.get("KB_SKIP_MAIN"))


def _prep(edge_index):
    """Host-side graph prep: per-core chunked edge layout + uniform plan."""
    src = np.asarray(edge_index[0]).astype(np.int64)
    dst = np.asarray(edge_index[1]).astype(np.int64)

    deg = np.bincount(dst, minlength=N).astype(np.float64) + 1.0
    dinv = (1.0 / np.sqrt(deg)).astype(np.float32)

    per_core = []
    cnt = np.zeros((NCORES, 2, NB), dtype=np.int64)
    for c in range(NCORES):
        lo, hi = c * NPC, (c + 1) * NPC
        m = (dst >= lo) & (dst < hi)
        es = src[m]
        dl = dst[m] - lo
        g = (es >= GRP).astype(np.int64)
        blk = dl >> 7
        slot = dl & 127
        # sort by (group, block, src) - src order improves HBM locality
        order = np.lexsort((es, blk, g))
        es, slot, g, blk = es[order], slot[order], g[order], blk[order]
        np.add.at(cnt[c], (g, blk), 1)
        per_core.append((es, slot, g, blk))

    # uniform chunk counts per (group, block) across cores (SPMD program)
    ct = np.ceil(cnt.max(axis=0) / 128.0).astype(np.int64)  # [2, NB]
    CH = ct.sum(axis=1)                                     # chunks per group
    off = np.zeros((2, NB), dtype=np.int64)
    off[:, 1:] = np.cumsum(ct, axis=1)[:, :-1]

    def wrap_idx(loc):
        nch = len(loc) // 128
        cols = []
        for s0 in range(0, nch, SEGC):
            seg = loc[s0 * 128: min(nch, s0 + SEGC) * 128]
            a = seg.reshape(-1, 16).T.astype(np.int16)        # [16, cols_s]
            cols.append(np.tile(a, (8, 1)))                   # [128, cols_s]
        return np.concatenate(cols, axis=1)

    in_extra = []
    for c in range(NCORES):
        es, slot, gs, blks = per_core[c]
        d = {}
        for g in range(2):
            loc = np.zeros(CH[g] * 128, dtype=np.int64)
            slo = np.full(CH[g] * 128, PADSLOT, dtype=np.float64)
            sel = gs == g
            ep, sl, bl = es[sel] - g * GRP, slot[sel], blks[sel]
            bstart = np.zeros(NB, dtype=np.int64)
            bstart[1:] = np.cumsum(np.bincount(bl, minlength=NB))[:-1]
            pos = off[g][bl] * 128 + (np.arange(len(ep)) - bstart[bl])
            loc[pos] = ep
            slo[pos] = sl
            d[f"idx{g}"] = wrap_idx(loc)
            d[f"dsb{g}"] = slo.reshape(-1, 128).T.astype(np.float32)
        in_extra.append(d)

    plan = {"ct": ct, "CH": CH, "off": off}
    return dinv, in_extra, plan


def _build(plan, bias_info):
    import concourse.bacc as bacc
    import concourse.tile as tile
    from concourse import mybir

    f32 = mybir.dt.float32
    f16 = mybir.dt.bfloat16
    i16 = mybir.dt.int16
    Alu = mybir.AluOpType
    Act = mybir.ActivationFunctionType

    ct, CH, off = plan["ct"], plan["CH"], plan["off"]
    has_bg, has_b1, has_b2, b3val = bias_info

    nc = bacc.Bacc("TRN2", target_bir_lowering=False, debug=False,
                   num_devices=NCORES,
                   dynamic_dma_scratch_size=DDS)

    def din(name, shape, dt=f32):
        return nc.dram_tensor(name, list(shape), dt, kind="ExternalInput")

    xdq_d = din("xdq", [N, D], f16)          # dinv*x, full table (bf16)
    xdT_d = din("xdT", [128, NPAD], f16)     # (dinv*x)[own].T
    xT_d = din("xT", [128, NPAD], f16)       # x[own].T (for residual colsum)
    dinvc_d = din("dinvc", [128, NB])
    iota_d = din("iotam", [128, 128], f16)   # each row = 0..127
    idh_d = din("idh", [128, 128], f16)
    idf_d = din("idf", [128, 128])
    ones_d = din("onesh", [128, 1], f16)
    wgT_d = din("wgT", [D, D], f16)          # Wg.T
    w1T_d = din("w1T", [128, 512])           # W1.T
    w2tp_d = din("w2tp", [128, 1024])        # W2.T row-blocks packed
    w3c_d = din("w3c", [128, 2])             # W3.T column chunks
    idx_d = [din(f"idx{g}", [128, int(CH[g]) * 8], i16) for g in range(2)]
    dsb_d = [din(f"dsb{g}", [128, int(CH[g])]) for g in range(2)]
    bgr_d = din("bgr", [1, 128]) if has_bg else None
    b1c_d = din("b1c", [128, 4]) if has_b1 else None
    b2c_d = din("b2c", [128, 2]) if has_b2 else None
    out_d = nc.dram_tensor("out", [1, 1], f32, kind="ExternalOutput")

    vb = nc.dram_tensor("vb", [1, 128], f32)
    vr = nc.dram_tensor("vr", [NCORES, 128], f32, addr_space="Shared")

    RG = [list(range(NCORES))]

    with tile.TileContext(nc) as tc:
        with (
            tc.tile_pool(name="const", bufs=1) as cpool,
            tc.tile_pool(name="seg", bufs=8) as segpool,
            tc.tile_pool(name="oh", bufs=8) as ohpool,
            tc.tile_pool(name="agg", bufs=3) as apool,
            tc.tile_pool(name="hb", bufs=3) as hpool,
            tc.tile_pool(name="mlp", bufs=1) as mpool,
            tc.tile_pool(name="psv", bufs=1, space="PSUM") as pvpool,
            tc.tile_pool(name="psS", bufs=3, space="PSUM") as pspool,
            tc.tile_pool(name="psH", bufs=2, space="PSUM") as phpool,
            tc.tile_pool(name="psT", bufs=1, space="PSUM") as ptpool,
        ):
            # ---- prefetch: gather-critical tables first ----
            # idx0 split so the first gather segment unblocks early
            idx_t = []
            dsb_t = []
            it0 = cpool.tile([128, int(CH[0]) * 8], i16, tag="idx0")
            c_split = min(4 * SEGC * 8, int(CH[0]) * 8)
            nc.sync.dma_start(it0[:, 0:c_split], idx_d[0][:, 0:c_split])
            dt0 = cpool.tile([128, int(CH[0])], f32, tag="dsb0")
            nc.sync.dma_start(dt0[:], dsb_d[0][:])
            iota_t = cpool.tile([128, 128], f16)
            nc.sync.dma_start(iota_t[:], iota_d[:])
            nc.sync.dma_start(it0[:, c_split:int(CH[0]) * 8],
                              idx_d[0][:, c_split:int(CH[0]) * 8])
            idx_t.append(it0)
            dsb_t.append(dt0)
            it1 = cpool.tile([128, int(CH[1]) * 8], i16, tag="idx1")
            nc.sync.dma_start(it1[:], idx_d[1][:])
            idx_t.append(it1)
            dt1 = cpool.tile([128, int(CH[1])], f32, tag="dsb1")
            nc.sync.dma_start(dt1[:], dsb_d[1][:])
            dsb_t.append(dt1)
            dinv_t = cpool.tile([128, NB], f32)
            nc.sync.dma_start(dinv_t[:], dinvc_d[:])
            ones_t = cpool.tile([128, 1], f16)
            nc.sync.dma_start(ones_t[:], ones_d[:])
            wgT_t = cpool.tile([128, 128], f16, tag="wgT")
            nc.sync.dma_start(wgT_t[:], wgT_d[:])
            xdT_t = cpool.tile([128, NPAD], f16, tag="xdT")
            nc.sync.dma_start(xdT_t[:], xdT_d[:])
            # late-use tiles; loads issued after the block loop below
            xT_t = cpool.tile([128, NPAD], f16, tag="xT")
            idh_t = cpool.tile([128, 128], f16)
            idf_t = cpool.tile([128, 128], f32)

            if has_bg:
                bgr_t = cpool.tile([1, 128], f32, tag="bgr")
                nc.sync.dma_start(bgr_t[:], bgr_d[:])
                # materialize bg as [128,128] via ones-outer-product
                ones_f = cpool.tile([128, 1], f32, tag="onesf")
                nc.vector.memset(ones_f[:], 1.0)
                psb = ptpool.tile([128, 128], f32, tag="pst")
                nc.tensor.matmul(psb[:], ones_f[:], bgr_t[:],
                                 start=True, stop=True)
                bg_t = cpool.tile([128, 128], f32, tag="bgt")
                nc.vector.tensor_copy(bg_t[:], psb[:])

            # ---- main: gather + one-hot segment-sum + W apply ----
            ytab = [xdq_d[0:GRP, :], xdq_d[GRP:N, :]]
            nseg = [int(np.ceil(CH[g] / SEGC)) for g in range(2)]
            seg_tiles = [[None] * nseg[g] for g in range(2)]
            oh_tiles = [[None] * nseg[g] for g in range(2)]
            seg_ptr = [0, 0]

            def ensure_seg(g, s):
                while seg_ptr[g] <= s:
                    si = seg_ptr[g]
                    ncols = min(SEGC, int(CH[g]) - si * SEGC)
                    tl = segpool.tile([128, ncols, 128], f16, tag="seg")
                    nidx = ncols * 128
                    nc.gpsimd.dma_gather(
                        tl[:], ytab[g], idx_t[g][:, si * (SEGC * 8):
                                                 si * (SEGC * 8) + ncols * 8],
                        num_idxs=nidx, num_idxs_reg=nidx,
                        elem_size=128, elem_step=128)
                    seg_tiles[g][si] = tl
                    oh = ohpool.tile([128, ncols, 128], f16, tag="oh")
                    for cc in range(ncols):
                        nc.vector.tensor_scalar(
                            oh[:, cc, :], iota_t[:],
                            dsb_t[g][:, si * SEGC + cc: si * SEGC + cc + 1],
                            None, Alu.is_equal)
                    oh_tiles[g][si] = oh
                    seg_ptr[g] += 1

            psv = pvpool.tile([1, 128], f32, tag="psv")
            first_v = [True]
            nblk = (0 if SKIP_MAIN else
                    (NB if DEBUG_BLOCKS is None else DEBUG_BLOCKS))
            for b in range(nblk):
                psS = pspool.tile([128, 128], f32, tag="psS")
                tot = int(ct[0][b] + ct[1][b])
                k = 0
                for g in range(2):
                    for j in range(int(ct[g][b])):
                        ci = int(off[g][b]) + j
                        s, col = divmod(ci, SEGC)
                        ensure_seg(g, s)
                        nc.tensor.matmul(psS[:], seg_tiles[g][s][:, col, :],
                                         oh_tiles[g][s][:, col, :],
                                         start=(k == 0), stop=(k == tot - 1))
                        k += 1
                # aggT: [fin, slot] PSUM -> SBUF bf16 (ACT engine)
                aggT = apool.tile([128, 128], f16, tag="agg")
                if tot == 0:
                    nc.vector.memset(aggT[:], 0.0)
                else:
                    nc.scalar.activation(aggT[:], psS[:], Act.Copy)
                # W apply + self-loop term
                psH = phpool.tile([128, 128], f32, tag="psH")
                nc.tensor.matmul(psH[:], aggT[:], wgT_t[:],
                                 start=True, stop=False)
                nc.tensor.matmul(psH[:], xdT_t[:, b * 128:(b + 1) * 128],
                                 wgT_t[:], start=False, stop=True)
                hb = hpool.tile([128, 128], f16)
                if has_bg:
                    tmp = hpool.tile([128, 128], f32, tag="tmp")
                    nc.vector.tensor_scalar(tmp[:], psH[:],
                                            dinv_t[:, b:b + 1], None, Alu.mult)
                    nc.vector.tensor_tensor(tmp[:], tmp[:], bg_t[:], Alu.add)
                    nc.scalar.activation(hb[:], tmp[:], Act.Relu)
                    if b == NB - 1:
                        nc.vector.memset(hb[NPC - (NB - 1) * 128:128, :], 0.0)
                else:
                    nc.scalar.activation(hb[:], psH[:], Act.Relu,
                                         scale=dinv_t[:, b:b + 1])
                nc.tensor.matmul(psv[:], ones_t[:], hb[:],
                                 start=first_v[0], stop=False,
                                 skip_group_check=True)
                first_v[0] = False

            # late-use loads (overlap the tail of the gather stream)
            nc.sync.dma_start(xT_t[:], xT_d[:])
            nc.sync.dma_start(idh_t[:], idh_d[:])
            nc.sync.dma_start(idf_t[:], idf_d[:])
            w1T_t = mpool.tile([128, 512], f32, tag="w1T")
            nc.sync.dma_start(w1T_t[:], w1T_d[:])
            w2tp_t = mpool.tile([128, 1024], f32, tag="w2tp")
            nc.sync.dma_start(w2tp_t[:], w2tp_d[:])
            w3c_t = mpool.tile([128, 2], f32, tag="w3c")
            nc.sync.dma_start(w3c_t[:], w3c_d[:])
            if has_b1:
                b1c_t = mpool.tile([128, 4], f32, tag="b1c")
                nc.sync.dma_start(b1c_t[:], b1c_d[:])
            if has_b2:
                b2c_t = mpool.tile([128, 2], f32, tag="b2c")
                nc.sync.dma_start(b2c_t[:], b2c_d[:])

            # residual colsum: vx[f] = sum_own x  (reduce over free dim)
            vx = cpool.tile([128, 1], f32, tag="vx")
            nc.vector.tensor_reduce(vx[:], xT_t[:], mybir.AxisListType.X,
                                    Alu.add)
            vxh = cpool.tile([128, 1], f16, tag="vxh")
            nc.vector.tensor_copy(vxh[:], vx[:])
            nc.tensor.matmul(psv[:], vxh[:], idh_t[:],
                             start=first_v[0], stop=True,
                             skip_group_check=True)

            # ---- v exchange: AllGather partial rows, sum on-chip ----
            vrow = mpool.tile([1, 128], f32, tag="vrow")
            nc.scalar.copy(vrow[:], psv[:])
            nc.sync.dma_start(vb[:], vrow[:])
            nc.gpsimd.collective_compute(
                "AllGather", Alu.bypass, replica_groups=RG,
                ins=[vb[:]], outs=[vr[:]])
            vgt = mpool.tile([NCORES, 128], f32, tag="vgt")
            nc.sync.dma_start(vgt[:], vr[:])
            ones8 = mpool.tile([NCORES, 1], f32, tag="ones8")
            nc.vector.memset(ones8[:], 1.0)
            psv2 = pvpool.tile([1, 128], f32, tag="psv")
            nc.tensor.matmul(psv2[:], ones8[:], vgt[:], start=True, stop=True)
            vfull = mpool.tile([1, 128], f32, tag="vfull")
            nc.scalar.copy(vfull[:], psv2[:])

            # ---- MLP head ----
            if SKIP_MLP:
                nc.sync.dma_start(out_d[:], vfull[0:1, 0:1])
            else:
                pst = ptpool.tile([128, 1], f32, tag="pst")
                nc.tensor.transpose(pst[:], vfull[:], idf_t[0:1, 0:1])
                vcol = mpool.tile([128, 1], f32, tag="vcol")
                nc.vector.tensor_copy(vcol[:], pst[:])

                a1c = []
                for m in range(4):
                    ps1 = ptpool.tile([128, 1], f32, tag="pst")
                    nc.tensor.matmul(ps1[:], w1T_t[:, m * 128:(m + 1) * 128],
                                     vcol[:], start=True, stop=True)
                    a1 = mpool.tile([128, 1], f32, tag=f"a1c{m}")
                    if has_b1:
                        nc.scalar.activation(a1[:], ps1[:], Act.Relu,
                                             bias=b1c_t[:, m:m + 1])
                    else:
                        nc.scalar.activation(a1[:], ps1[:], Act.Relu)
                    a1c.append(a1)

                a2c = []
                for m in range(2):
                    ps2 = ptpool.tile([128, 1], f32, tag="pst")
                    for kk in range(4):
                        nc.tensor.matmul(
                            ps2[:],
                            w2tp_t[:, kk * 256 + m * 128:
                                   kk * 256 + (m + 1) * 128],
                            a1c[kk][:], start=(kk == 0), stop=(kk == 3))
                    a2 = mpool.tile([128, 1], f32, tag=f"a2c{m}")
                    if has_b2:
                        nc.scalar.activation(a2[:], ps2[:], Act.Relu,
                                             bias=b2c_t[:, m:m + 1])
                    else:
                        nc.scalar.activation(a2[:], ps2[:], Act.Relu)
                    a2c.append(a2)

                ps3 = ptpool.tile([1, 1], f32, tag="ps3")
                for kk in range(2):
                    nc.tensor.matmul(ps3[:], w3c_t[:, kk:kk + 1], a2c[kk][:],
                                     start=(kk == 0), stop=(kk == 1))
                ot = mpool.tile([1, 1], f32, tag="ot")
                nc.scalar.activation(ot[:], ps3[:], Act.Copy,
                                     bias=float(b3val))
                nc.sync.dma_start(out_d[:], ot[:])

    nc.compile()
    return nc


TRACE = False
LAST_EXEC_NS = None
LAST_RESULT = None


def kernel(**inputs):
    from concourse.bass_utils import run_bass_kernel_spmd

    x = np.asarray(inputs["x"], dtype=np.float32)
    Wg = np.asarray(inputs["Wg"], dtype=np.float32)
    bg = np.asarray(inputs["bg"], dtype=np.float32)
    W1 = np.asarray(inputs["W1"], dtype=np.float32)
    b1 = np.asarray(inputs["b1"], dtype=np.float32)
    W2 = np.asarray(inputs["W2"], dtype=np.float32)
    b2 = np.asarray(inputs["b2"], dtype=np.float32)
    W3 = np.asarray(inputs["W3"], dtype=np.float32)
    b3 = np.asarray(inputs["b3"], dtype=np.float32)

    dinv, in_extra, plan = _prep(inputs["edge_index"])
    bias_info = (bool(bg.any()), bool(b1.any()), bool(b2.any()),
                 float(b3.reshape(-1)[0]))
    nc = _build(plan, bias_info)

    xd = (dinv[:, None] * x).astype(BF16)           # [N, D] table
    iota = np.tile(np.arange(128, dtype=np.float32)[None, :],
                   (128, 1)).astype(BF16)
    idh = np.eye(128).astype(BF16)
    idf = np.eye(128, dtype=np.float32)
    onesh = np.ones((128, 1), dtype=np.float32).astype(BF16)
    wgT = Wg.T.astype(BF16).copy()
    w1T = W1.T.astype(np.float32).copy()            # [128, 512]
    w2tp = np.concatenate([W2.T[k * 128:(k + 1) * 128] for k in range(4)],
                          axis=1).astype(np.float32).copy()  # [128, 1024]
    w3c = W3.reshape(2, 128).T.astype(np.float32).copy()     # [128, 2]

    in_maps = []
    for c in range(NCORES):
        lo, hi = c * NPC, (c + 1) * NPC
        xdT = np.zeros((128, NPAD), dtype=BF16)
        xdT[:, :NPC] = xd[lo:hi].T
        xT = np.zeros((128, NPAD), dtype=BF16)
        xT[:, :NPC] = x[lo:hi].T.astype(BF16)
        dv = np.zeros(NPAD, dtype=np.float32)
        dv[:NPC] = dinv[lo:hi]
        dvc = dv.reshape(NB, 128).T.copy()
        m = {"xdq": xd, "xdT": xdT, "xT": xT, "dinvc": dvc,
             "iotam": iota, "idh": idh, "idf": idf, "onesh": onesh,
             "wgT": wgT, "w1T": w1T, "w2tp": w2tp, "w3c": w3c,
             "idx0": in_extra[c]["idx0"], "idx1": in_extra[c]["idx1"],
             "dsb0": in_extra[c]["dsb0"], "dsb1": in_extra[c]["dsb1"]}
        if bias_info[0]:
            m["bgr"] = bg.reshape(1, 128)
        if bias_info[1]:
            m["b1c"] = b1.reshape(4, 128).T.astype(np.float32).copy()
        if bias_info[2]:
            m["b2c"] = b2.reshape(2, 128).T.astype(np.float32).copy()
        in_maps.append(m)

    res = run_bass_kernel_spmd(nc, in_maps, list(range(NCORES)), trace=TRACE)
    global LAST_EXEC_NS, LAST_RESULT
    LAST_EXEC_NS = res.exec_time_ns
    LAST_RESULT = res
    return res.results[0]["out"].reshape(1).astype(np.float32)


# revision 29
# speedup vs baseline: 1.0213x; 1.0213x over previous
"""GCN critic network kernel for 8 TRN2 NeuronCores.

Strategy (dst-shard, no message-table collective):
  The GCNConv linear commutes with the segment sum:
      out[d] = dinv_d * (sum_{s in N(d)} dinv_s * x[s]) @ Wg.T + bg
  so each core gathers pre-scaled raw rows xd = dinv*x (host-prepped bf16
  table in DRAM, a kernel input) for its own dst shard's edges and applies
  Wg once per 128-dst block after aggregation. This removes the y-table
  AllGather of the previous design entirely; the only collective left is
  the tiny [1,128] AllReduce of the pooled vector.

  - dst nodes sharded 6250/core (padded 6272 = 49 blocks of 128).
  - Edge messages: dma_gather of 256B bf16 rows (int16 indices, node table
    split in two <=32768-row groups), segment-summed per dst block via
    one-hot matmul accumulation in PSUM:  psST[fin, slot] += seg.T @ oh.
  - One-hot built per chunk with a single DVE tensor_scalar is_equal
    (iota row constant vs per-partition dst-slot scalar) - hits the
    packed-2-byte DVE fast path.
  - Self-loops folded in as one extra matmul per block from the resident
    xdT slice (no gathered self rows).
  - h = relu(dinv * (aggT.T @ WgT)); v = ones@h blocks + colsum(x_own);
    AllReduce v; tiny MLP head with host-pretransposed weights.
"""

import os
import numpy as np
import ml_dtypes

BF16 = ml_dtypes.bfloat16
N = 50000
E = 800000
D = 128
NCORES = 8
NPC = 6250          # dst nodes per core
NPAD = 6272         # padded (49 * 128)
NB = NPAD // 128    # dst blocks per core
GRP = 32768         # int16 index-group boundary (table-row space)
SEGC = int(os.environ.get("KB_SEGC", "8"))   # chunks per gather call
DDS = int(os.environ.get("KB_DDS", "65536"))
PADSLOT = 300.0     # dst-slot sentinel for padding rows (one-hot miss)

DEBUG_BLOCKS = (int(os.environ["KB_DEBUG_BLOCKS"])
                if "KB_DEBUG_BLOCKS" in os.environ else None)
SKIP_MLP = bool(os.environ.get("KB_SKIP_MLP"))
SKIP_MAIN = bool(os.environ.get("KB_SKIP_MAIN"))


def _lpt_assign(src, dst):
    """Per-core 2D-LPT node->(block, slot) map balancing per-(group, block)
    edge counts across cores; shrinks the uniform chunk-count padding."""
    c0 = np.bincount(dst[src < GRP], minlength=N)
    c1 = np.bincount(dst[src >= GRP], minlength=N)
    node_blk = np.empty(N, np.int64)
    node_slot = np.empty(N, np.int64)
    for c in range(NCORES):
        nodes = np.arange(c * NPC, (c + 1) * NPC)
        a0, a1 = c0[nodes].astype(np.float64), c1[nodes].astype(np.float64)
        order = np.argsort(-(a0 + a1), kind="stable")
        T0 = max(a0.sum() / NB, 1.0)
        T1 = max(a1.sum() / NB, 1.0)
        w0 = np.zeros(NB)
        w1 = np.zeros(NB)
        ns = np.zeros(NB, np.int64)
        for j in order:
            pen = np.where(ns < 128,
                           np.maximum((w0 + a0[j]) / T0, (w1 + a1[j]) / T1),
                           np.inf)
            b = int(np.argmin(pen))
            node_blk[nodes[j]] = b
            node_slot[nodes[j]] = ns[b]
            w0[b] += a0[j]
            w1[b] += a1[j]
            ns[b] += 1
    return node_blk, node_slot


def _prep(edge_index, use_lpt=True):
    """Host-side graph prep: per-core chunked edge layout + uniform plan."""
    src = np.asarray(edge_index[0]).astype(np.int64)
    dst = np.asarray(edge_index[1]).astype(np.int64)

    deg = np.bincount(dst, minlength=N).astype(np.float64) + 1.0
    dinv = (1.0 / np.sqrt(deg)).astype(np.float32)

    if use_lpt and not os.environ.get("KB_NOLPT"):
        node_blk, node_slot = _lpt_assign(src, dst)
    else:
        local = np.arange(N) % NPC
        node_blk = local >> 7
        node_slot = local & 127

    per_core = []
    cnt = np.zeros((NCORES, 2, NB), dtype=np.int64)
    for c in range(NCORES):
        lo, hi = c * NPC, (c + 1) * NPC
        m = (dst >= lo) & (dst < hi)
        es = src[m]
        ds = dst[m]
        g = (es >= GRP).astype(np.int64)
        blk = node_blk[ds]
        slot = node_slot[ds]
        # sort by (group, block, src) - src order improves HBM locality
        order = np.lexsort((es, blk, g))
        es, slot, g, blk = es[order], slot[order], g[order], blk[order]
        np.add.at(cnt[c], (g, blk), 1)
        per_core.append((es, slot, g, blk))

    # uniform chunk counts per (group, block) across cores (SPMD program)
    ct = np.ceil(cnt.max(axis=0) / 128.0).astype(np.int64)  # [2, NB]
    CH = ct.sum(axis=1)                                     # chunks per group
    off = np.zeros((2, NB), dtype=np.int64)
    off[:, 1:] = np.cumsum(ct, axis=1)[:, :-1]

    def wrap_idx(loc):
        nch = len(loc) // 128
        cols = []
        for s0 in range(0, nch, SEGC):
            seg = loc[s0 * 128: min(nch, s0 + SEGC) * 128]
            a = seg.reshape(-1, 16).T.astype(np.int16)        # [16, cols_s]
            cols.append(np.tile(a, (8, 1)))                   # [128, cols_s]
        return np.concatenate(cols, axis=1)

    in_extra = []
    for c in range(NCORES):
        es, slot, gs, blks = per_core[c]
        d = {}
        for g in range(2):
            loc = np.zeros(CH[g] * 128, dtype=np.int64)
            slo = np.full(CH[g] * 128, PADSLOT, dtype=np.float64)
            sel = gs == g
            ep, sl, bl = es[sel] - g * GRP, slot[sel], blks[sel]
            bstart = np.zeros(NB, dtype=np.int64)
            bstart[1:] = np.cumsum(np.bincount(bl, minlength=NB))[:-1]
            pos = off[g][bl] * 128 + (np.arange(len(ep)) - bstart[bl])
            loc[pos] = ep
            slo[pos] = sl
            d[f"idx{g}"] = wrap_idx(loc)
            d[f"dsb{g}"] = slo.reshape(-1, 128).T.astype(BF16)
        in_extra.append(d)

    plan = {"ct": ct, "CH": CH, "off": off,
            "nblk": node_blk, "nslot": node_slot}
    return dinv, in_extra, plan


def core_tables(c, x, xd, dinv, plan):
    """Per-core dst-side tables in the (possibly LPT-permuted) layout."""
    nblk, nslot = plan["nblk"], plan["nslot"]
    lo, hi = c * NPC, (c + 1) * NPC
    pos = nblk[lo:hi] * 128 + nslot[lo:hi]
    xdT = np.zeros((128, NPAD), dtype=BF16)
    xdT[:, pos] = xd[lo:hi].T
    xT = np.zeros((128, NPAD), dtype=BF16)
    xT[:, pos] = x[lo:hi].T.astype(BF16)
    dv = np.zeros(NPAD, dtype=np.float32)
    dv[pos] = dinv[lo:hi]
    dvc = dv.reshape(NB, 128).T.copy()
    return xdT, xT, dvc


def _build(plan, bias_info):
    import concourse.bacc as bacc
    import concourse.tile as tile
    from concourse import mybir

    f32 = mybir.dt.float32
    f16 = mybir.dt.bfloat16
    i16 = mybir.dt.int16
    Alu = mybir.AluOpType
    Act = mybir.ActivationFunctionType

    ct, CH, off = plan["ct"], plan["CH"], plan["off"]
    has_bg, has_b1, has_b2, b3val = bias_info

    nc = bacc.Bacc("TRN2", target_bir_lowering=False, debug=False,
                   num_devices=NCORES,
                   dynamic_dma_scratch_size=DDS)

    def din(name, shape, dt=f32):
        return nc.dram_tensor(name, list(shape), dt, kind="ExternalInput")

    xdq_d = din("xdq", [N, D], f16)          # dinv*x, full table (bf16)
    xdT_d = din("xdT", [128, NPAD], f16)     # (dinv*x)[own].T
    xT_d = din("xT", [128, NPAD], f16)       # x[own].T (for residual colsum)
    dinvc_d = din("dinvc", [128, NB])
    wgT_d = din("wgT", [D, D], f16)          # Wg.T
    w1T_d = din("w1T", [128, 512])           # W1.T
    w2tp_d = din("w2tp", [128, 1024])        # W2.T row-blocks packed
    w3c_d = din("w3c", [128, 2])             # W3.T column chunks
    idx_d = [din(f"idx{g}", [128, int(CH[g]) * 8], i16) for g in range(2)]
    dsb_d = [din(f"dsb{g}", [128, int(CH[g])], f16) for g in range(2)]
    bgr_d = din("bgr", [1, 128]) if has_bg else None
    b1c_d = din("b1c", [128, 4]) if has_b1 else None
    b2c_d = din("b2c", [128, 2]) if has_b2 else None
    out_d = nc.dram_tensor("out", [1, 1], f32, kind="ExternalOutput")

    vb = nc.dram_tensor("vb", [1, 128], f32)
    vr = nc.dram_tensor("vr", [NCORES, 128], f32, addr_space="Shared")

    RG = [list(range(NCORES))]

    with tile.TileContext(nc) as tc:
        with (
            tc.tile_pool(name="const", bufs=1) as cpool,
            tc.tile_pool(name="seg", bufs=10) as segpool,
            tc.tile_pool(name="oh", bufs=10) as ohpool,
            tc.tile_pool(name="agg", bufs=3) as apool,
            tc.tile_pool(name="hb", bufs=3) as hpool,
            tc.tile_pool(name="mlp", bufs=1) as mpool,
            tc.tile_pool(name="psv", bufs=1, space="PSUM") as pvpool,
            tc.tile_pool(name="psS", bufs=3, space="PSUM") as pspool,
            tc.tile_pool(name="psH", bufs=2, space="PSUM") as phpool,
            tc.tile_pool(name="psT", bufs=1, space="PSUM") as ptpool,
        ):
            # ---- prefetch: gather-critical tables first ----
            # idx0 split so the first gather segment unblocks early
            idx_t = []
            it0 = cpool.tile([128, int(CH[0]) * 8], i16, tag="idx0")
            c_split = min(4 * SEGC * 8, int(CH[0]) * 8)
            nc.sync.dma_start(it0[:, 0:c_split], idx_d[0][:, 0:c_split])
            dsbh_t = []
            dh0 = cpool.tile([128, int(CH[0])], f16, tag="dsbh0")
            nc.sync.dma_start(dh0[:], dsb_d[0][:])
            nc.sync.dma_start(it0[:, c_split:int(CH[0]) * 8],
                              idx_d[0][:, c_split:int(CH[0]) * 8])
            idx_t.append(it0)
            dsbh_t.append(dh0)
            it1 = cpool.tile([128, int(CH[1]) * 8], i16, tag="idx1")
            nc.sync.dma_start(it1[:], idx_d[1][:])
            idx_t.append(it1)
            dh1 = cpool.tile([128, int(CH[1])], f16, tag="dsbh1")
            nc.sync.dma_start(dh1[:], dsb_d[1][:])
            dsbh_t.append(dh1)
            dinv_t = cpool.tile([128, NB], f32)
            nc.sync.dma_start(dinv_t[:], dinvc_d[:])
            wgT_t = cpool.tile([128, 128], f16, tag="wgT")
            nc.sync.dma_start(wgT_t[:], wgT_d[:])
            xdT_t = cpool.tile([128, NPAD], f16, tag="xdT")
            nc.sync.dma_start(xdT_t[:], xdT_d[:])

            # on-chip constants (no DMA): iota row, identities, ones
            iota_t = cpool.tile([128, 128], f16)
            nc.gpsimd.iota(iota_t[:], [[1, 128]], channel_multiplier=0,
                           allow_small_or_imprecise_dtypes=True)
            pcol_t = cpool.tile([128, 1], f32, tag="pcol")
            nc.gpsimd.iota(pcol_t[:], [[0, 1]], channel_multiplier=1,
                           allow_small_or_imprecise_dtypes=True)
            ones_t = cpool.tile([128, 1], f16)
            nc.vector.memset(ones_t[:], 1.0)
            # dsb tables cast to f32 (is_equal scalar operand must be f32)
            dsb_t = []
            for g in range(2):
                dt_ = cpool.tile([128, int(CH[g])], f32, tag=f"dsb{g}")
                nc.vector.tensor_copy(dt_[:], dsbh_t[g][:])
                dsb_t.append(dt_)
            idh_t = cpool.tile([128, 128], f16)
            nc.vector.tensor_scalar(idh_t[:], iota_t[:], pcol_t[:],
                                    None, Alu.is_equal)
            idf_t = cpool.tile([128, 128], f32)
            nc.vector.tensor_scalar(idf_t[:], iota_t[:], pcol_t[:],
                                    None, Alu.is_equal)
            # late-use tile; load issued after the block loop below
            xT_t = cpool.tile([128, NPAD], f16, tag="xT")

            if has_bg:
                bgr_t = cpool.tile([1, 128], f32, tag="bgr")
                nc.sync.dma_start(bgr_t[:], bgr_d[:])
                # materialize bg as [128,128] via ones-outer-product
                ones_f = cpool.tile([128, 1], f32, tag="onesf")
                nc.vector.memset(ones_f[:], 1.0)
                psb = ptpool.tile([128, 128], f32, tag="pst")
                nc.tensor.matmul(psb[:], ones_f[:], bgr_t[:],
                                 start=True, stop=True)
                bg_t = cpool.tile([128, 128], f32, tag="bgt")
                nc.vector.tensor_copy(bg_t[:], psb[:])

            # ---- main: gather + one-hot segment-sum + W apply ----
            ytab = [xdq_d[0:GRP, :], xdq_d[GRP:N, :]]
            nseg = [int(np.ceil(CH[g] / SEGC)) for g in range(2)]
            seg_tiles = [[None] * nseg[g] for g in range(2)]
            oh_tiles = [[None] * nseg[g] for g in range(2)]
            seg_ptr = [0, 0]

            def ensure_seg(g, s):
                while seg_ptr[g] <= s:
                    si = seg_ptr[g]
                    ncols = min(SEGC, int(CH[g]) - si * SEGC)
                    tl = segpool.tile([128, ncols, 128], f16, tag="seg")
                    nidx = ncols * 128
                    nc.gpsimd.dma_gather(
                        tl[:], ytab[g], idx_t[g][:, si * (SEGC * 8):
                                                 si * (SEGC * 8) + ncols * 8],
                        num_idxs=nidx, num_idxs_reg=nidx,
                        elem_size=128, elem_step=128)
                    seg_tiles[g][si] = tl
                    oh = ohpool.tile([128, ncols, 128], f16, tag="oh")
                    for cc in range(ncols):
                        nc.vector.tensor_scalar(
                            oh[:, cc, :], iota_t[:],
                            dsb_t[g][:, si * SEGC + cc: si * SEGC + cc + 1],
                            None, Alu.is_equal)
                    oh_tiles[g][si] = oh
                    seg_ptr[g] += 1

            psv = pvpool.tile([1, 128], f32, tag="psv")
            first_v = [True]
            nblk = (0 if SKIP_MAIN else
                    (NB if DEBUG_BLOCKS is None else DEBUG_BLOCKS))
            for b in range(nblk):
                psS = pspool.tile([128, 128], f32, tag="psS")
                tot = int(ct[0][b] + ct[1][b])
                k = 0
                for g in range(2):
                    for j in range(int(ct[g][b])):
                        ci = int(off[g][b]) + j
                        s, col = divmod(ci, SEGC)
                        ensure_seg(g, s)
                        nc.tensor.matmul(psS[:], seg_tiles[g][s][:, col, :],
                                         oh_tiles[g][s][:, col, :],
                                         start=(k == 0), stop=(k == tot - 1))
                        k += 1
                # aggT: [fin, slot] PSUM -> SBUF bf16 (ACT engine)
                aggT = apool.tile([128, 128], f16, tag="agg")
                if tot == 0:
                    nc.vector.memset(aggT[:], 0.0)
                else:
                    nc.scalar.activation(aggT[:], psS[:], Act.Copy)
                # W apply + self-loop term
                psH = phpool.tile([128, 128], f32, tag="psH")
                nc.tensor.matmul(psH[:], aggT[:], wgT_t[:],
                                 start=True, stop=False)
                nc.tensor.matmul(psH[:], xdT_t[:, b * 128:(b + 1) * 128],
                                 wgT_t[:], start=False, stop=True)
                hb = hpool.tile([128, 128], f16)
                if has_bg:
                    tmp = hpool.tile([128, 128], f32, tag="tmp")
                    nc.vector.tensor_scalar(tmp[:], psH[:],
                                            dinv_t[:, b:b + 1], None, Alu.mult)
                    nc.vector.tensor_tensor(tmp[:], tmp[:], bg_t[:], Alu.add)
                    nc.scalar.activation(hb[:], tmp[:], Act.Relu)
                    if b == NB - 1:
                        nc.vector.memset(hb[NPC - (NB - 1) * 128:128, :], 0.0)
                else:
                    nc.scalar.activation(hb[:], psH[:], Act.Relu,
                                         scale=dinv_t[:, b:b + 1])
                nc.tensor.matmul(psv[:], ones_t[:], hb[:],
                                 start=first_v[0], stop=False,
                                 skip_group_check=True)
                first_v[0] = False

            # late-use loads (overlap the tail of the gather stream)
            nc.sync.dma_start(xT_t[:], xT_d[:])
            w1T_t = mpool.tile([128, 512], f32, tag="w1T")
            nc.sync.dma_start(w1T_t[:], w1T_d[:])
            w2tp_t = mpool.tile([128, 1024], f32, tag="w2tp")
            nc.sync.dma_start(w2tp_t[:], w2tp_d[:])
            w3c_t = mpool.tile([128, 2], f32, tag="w3c")
            nc.sync.dma_start(w3c_t[:], w3c_d[:])
            if has_b1:
                b1c_t = mpool.tile([128, 4], f32, tag="b1c")
                nc.sync.dma_start(b1c_t[:], b1c_d[:])
            if has_b2:
                b2c_t = mpool.tile([128, 2], f32, tag="b2c")
                nc.sync.dma_start(b2c_t[:], b2c_d[:])

            # residual colsum: vx[f] = sum_own x  (reduce over free dim)
            vx = cpool.tile([128, 1], f32, tag="vx")
            nc.vector.tensor_reduce(vx[:], xT_t[:], mybir.AxisListType.X,
                                    Alu.add)
            vxh = cpool.tile([128, 1], f16, tag="vxh")
            nc.vector.tensor_copy(vxh[:], vx[:])
            nc.tensor.matmul(psv[:], vxh[:], idh_t[:],
                             start=first_v[0], stop=True,
                             skip_group_check=True)

            # ---- v exchange: AllGather partial rows, sum on-chip ----
            vrow = mpool.tile([1, 128], f32, tag="vrow")
            nc.scalar.copy(vrow[:], psv[:])
            nc.sync.dma_start(vb[:], vrow[:])
            nc.gpsimd.collective_compute(
                "AllGather", Alu.bypass, replica_groups=RG,
                ins=[vb[:]], outs=[vr[:]])
            vgt = mpool.tile([NCORES, 128], f32, tag="vgt")
            nc.sync.dma_start(vgt[:], vr[:])
            ones8 = mpool.tile([NCORES, 1], f32, tag="ones8")
            nc.vector.memset(ones8[:], 1.0)
            psv2 = pvpool.tile([1, 128], f32, tag="psv")
            nc.tensor.matmul(psv2[:], ones8[:], vgt[:], start=True, stop=True)
            vfull = mpool.tile([1, 128], f32, tag="vfull")
            nc.scalar.copy(vfull[:], psv2[:])

            # ---- MLP head ----
            if SKIP_MLP:
                nc.sync.dma_start(out_d[:], vfull[0:1, 0:1])
            else:
                pst = ptpool.tile([128, 1], f32, tag="pst")
                nc.tensor.transpose(pst[:], vfull[:], idf_t[0:1, 0:1])
                vcol = mpool.tile([128, 1], f32, tag="vcol")
                nc.vector.tensor_copy(vcol[:], pst[:])

                a1c = []
                for m in range(4):
                    ps1 = ptpool.tile([128, 1], f32, tag="pst")
                    nc.tensor.matmul(ps1[:], w1T_t[:, m * 128:(m + 1) * 128],
                                     vcol[:], start=True, stop=True)
                    a1 = mpool.tile([128, 1], f32, tag=f"a1c{m}")
                    if has_b1:
                        nc.scalar.activation(a1[:], ps1[:], Act.Relu,
                                             bias=b1c_t[:, m:m + 1])
                    else:
                        nc.scalar.activation(a1[:], ps1[:], Act.Relu)
                    a1c.append(a1)

                a2c = []
                for m in range(2):
                    ps2 = ptpool.tile([128, 1], f32, tag="pst")
                    for kk in range(4):
                        nc.tensor.matmul(
                            ps2[:],
                            w2tp_t[:, kk * 256 + m * 128:
                                   kk * 256 + (m + 1) * 128],
                            a1c[kk][:], start=(kk == 0), stop=(kk == 3))
                    a2 = mpool.tile([128, 1], f32, tag=f"a2c{m}")
                    if has_b2:
                        nc.scalar.activation(a2[:], ps2[:], Act.Relu,
                                             bias=b2c_t[:, m:m + 1])
                    else:
                        nc.scalar.activation(a2[:], ps2[:], Act.Relu)
                    a2c.append(a2)

                ps3 = ptpool.tile([1, 1], f32, tag="ps3")
                for kk in range(2):
                    nc.tensor.matmul(ps3[:], w3c_t[:, kk:kk + 1], a2c[kk][:],
                                     start=(kk == 0), stop=(kk == 1))
                ot = mpool.tile([1, 1], f32, tag="ot")
                nc.scalar.activation(ot[:], ps3[:], Act.Copy,
                                     bias=float(b3val))
                nc.sync.dma_start(out_d[:], ot[:])

    nc.compile()
    return nc


TRACE = False
LAST_EXEC_NS = None
LAST_RESULT = None


def kernel(**inputs):
    from concourse.bass_utils import run_bass_kernel_spmd

    x = np.asarray(inputs["x"], dtype=np.float32)
    Wg = np.asarray(inputs["Wg"], dtype=np.float32)
    bg = np.asarray(inputs["bg"], dtype=np.float32)
    W1 = np.asarray(inputs["W1"], dtype=np.float32)
    b1 = np.asarray(inputs["b1"], dtype=np.float32)
    W2 = np.asarray(inputs["W2"], dtype=np.float32)
    b2 = np.asarray(inputs["b2"], dtype=np.float32)
    W3 = np.asarray(inputs["W3"], dtype=np.float32)
    b3 = np.asarray(inputs["b3"], dtype=np.float32)

    bias_info = (bool(bg.any()), bool(b1.any()), bool(b2.any()),
                 float(b3.reshape(-1)[0]))
    # LPT layout leaves empty slots scattered across blocks, which the
    # bg!=0 path cannot mask; fall back to the sequential layout there.
    dinv, in_extra, plan = _prep(inputs["edge_index"],
                                 use_lpt=not bias_info[0])
    nc = _build(plan, bias_info)

    xd = (dinv[:, None] * x).astype(BF16)           # [N, D] table
    wgT = Wg.T.astype(BF16).copy()
    w1T = W1.T.astype(np.float32).copy()            # [128, 512]
    w2tp = np.concatenate([W2.T[k * 128:(k + 1) * 128] for k in range(4)],
                          axis=1).astype(np.float32).copy()  # [128, 1024]
    w3c = W3.reshape(2, 128).T.astype(np.float32).copy()     # [128, 2]

    in_maps = []
    for c in range(NCORES):
        xdT, xT, dvc = core_tables(c, x, xd, dinv, plan)
        m = {"xdq": xd, "xdT": xdT, "xT": xT, "dinvc": dvc,
             "wgT": wgT, "w1T": w1T, "w2tp": w2tp, "w3c": w3c,
             "idx0": in_extra[c]["idx0"], "idx1": in_extra[c]["idx1"],
             "dsb0": in_extra[c]["dsb0"], "dsb1": in_extra[c]["dsb1"]}
        if bias_info[0]:
            m["bgr"] = bg.reshape(1, 128)
        if bias_info[1]:
            m["b1c"] = b1.reshape(4, 128).T.astype(np.float32).copy()
        if bias_info[2]:
            m["b2c"] = b2.reshape(2, 128).T.astype(np.float32).copy()
        in_maps.append(m)

    res = run_bass_kernel_spmd(nc, in_maps, list(range(NCORES)), trace=TRACE)
    global LAST_EXEC_NS, LAST_RESULT
    LAST_EXEC_NS = res.exec_time_ns
    LAST_RESULT = res
    return res.results[0]["out"].reshape(1).astype(np.float32)


# revision 37
# speedup vs baseline: 1.0438x; 1.0220x over previous
"""GCN critic network kernel for 8 TRN2 NeuronCores.

Strategy (dst-shard, no message-table collective):
  The GCNConv linear commutes with the segment sum:
      out[d] = dinv_d * (sum_{s in N(d)} dinv_s * x[s]) @ Wg.T + bg
  so each core gathers pre-scaled raw rows xd = dinv*x (host-prepped bf16
  table in DRAM, a kernel input) for its own dst shard's edges and applies
  Wg once per 128-dst block after aggregation. This removes the y-table
  AllGather of the previous design entirely; the only collective left is
  the tiny [1,128] AllReduce of the pooled vector.

  - dst nodes sharded 6250/core (padded 6272 = 49 blocks of 128).
  - Edge messages: dma_gather of 256B bf16 rows (int16 indices, node table
    split in two <=32768-row groups), segment-summed per dst block via
    one-hot matmul accumulation in PSUM:  psST[fin, slot] += seg.T @ oh.
  - One-hot built per chunk with a single DVE tensor_scalar is_equal
    (iota row constant vs per-partition dst-slot scalar) - hits the
    packed-2-byte DVE fast path.
  - Self-loops folded in as one extra matmul per block from the resident
    xdT slice (no gathered self rows).
  - h = relu(dinv * (aggT.T @ WgT)); v = ones@h blocks + colsum(x_own);
    AllReduce v; tiny MLP head with host-pretransposed weights.
"""

import os
import numpy as np
import ml_dtypes

BF16 = ml_dtypes.bfloat16
N = 50000
E = 800000
D = 128
NCORES = 8
NPC = 6250          # dst nodes per core
NPAD = 6272         # padded (49 * 128)
NB = NPAD // 128    # dst blocks per core
GRP = 32768         # int16 index-group boundary (table-row space)
SEGC = int(os.environ.get("KB_SEGC", "8"))   # chunks per gather call
DDS = int(os.environ.get("KB_DDS", "65536"))
PADSLOT = 300.0     # dst-slot sentinel for padding rows (one-hot miss)

DEBUG_BLOCKS = (int(os.environ["KB_DEBUG_BLOCKS"])
                if "KB_DEBUG_BLOCKS" in os.environ else None)
SKIP_MLP = bool(os.environ.get("KB_SKIP_MLP"))
SKIP_MAIN = bool(os.environ.get("KB_SKIP_MAIN"))


def _lpt_assign(src, dst):
    """Per-core 2D-LPT node->(block, slot) map balancing per-(group, block)
    edge counts across cores; shrinks the uniform chunk-count padding."""
    c0 = np.bincount(dst[src < GRP], minlength=N)
    c1 = np.bincount(dst[src >= GRP], minlength=N)
    node_blk = np.empty(N, np.int64)
    node_slot = np.empty(N, np.int64)
    for c in range(NCORES):
        nodes = np.arange(c * NPC, (c + 1) * NPC)
        a0, a1 = c0[nodes].astype(np.float64), c1[nodes].astype(np.float64)
        order = np.argsort(-(a0 + a1), kind="stable")
        T0 = max(a0.sum() / NB, 1.0)
        T1 = max(a1.sum() / NB, 1.0)
        w0 = np.zeros(NB)
        w1 = np.zeros(NB)
        ns = np.zeros(NB, np.int64)
        for j in order:
            pen = np.where(ns < 128,
                           np.maximum((w0 + a0[j]) / T0, (w1 + a1[j]) / T1),
                           np.inf)
            b = int(np.argmin(pen))
            node_blk[nodes[j]] = b
            node_slot[nodes[j]] = ns[b]
            w0[b] += a0[j]
            w1[b] += a1[j]
            ns[b] += 1
    return node_blk, node_slot


def _prep(edge_index, use_lpt=True):
    """Host-side graph prep: per-core chunked edge layout + uniform plan."""
    src = np.asarray(edge_index[0]).astype(np.int64)
    dst = np.asarray(edge_index[1]).astype(np.int64)

    deg = np.bincount(dst, minlength=N).astype(np.float64) + 1.0
    dinv = (1.0 / np.sqrt(deg)).astype(np.float32)

    if use_lpt and not os.environ.get("KB_NOLPT"):
        node_blk, node_slot = _lpt_assign(src, dst)
    else:
        local = np.arange(N) % NPC
        node_blk = local >> 7
        node_slot = local & 127

    per_core = []
    cnt = np.zeros((NCORES, 2, NB), dtype=np.int64)
    for c in range(NCORES):
        lo, hi = c * NPC, (c + 1) * NPC
        m = (dst >= lo) & (dst < hi)
        es = src[m]
        ds = dst[m]
        g = (es >= GRP).astype(np.int64)
        blk = node_blk[ds]
        slot = node_slot[ds]
        # sort by (group, block, src) - src order improves HBM locality
        order = np.lexsort((es, blk, g))
        es, slot, g, blk = es[order], slot[order], g[order], blk[order]
        np.add.at(cnt[c], (g, blk), 1)
        per_core.append((es, slot, g, blk))

    # uniform chunk counts per (group, block) across cores (SPMD program)
    ct = np.ceil(cnt.max(axis=0) / 128.0).astype(np.int64)  # [2, NB]
    CH = ct.sum(axis=1)                                     # chunks per group
    off = np.zeros((2, NB), dtype=np.int64)
    off[:, 1:] = np.cumsum(ct, axis=1)[:, :-1]

    def wrap_idx(loc):
        nch = len(loc) // 128
        cols = []
        for s0 in range(0, nch, SEGC):
            seg = loc[s0 * 128: min(nch, s0 + SEGC) * 128]
            a = seg.reshape(-1, 16).T.astype(np.int16)        # [16, cols_s]
            cols.append(np.tile(a, (8, 1)))                   # [128, cols_s]
        return np.concatenate(cols, axis=1)

    in_extra = []
    for c in range(NCORES):
        es, slot, gs, blks = per_core[c]
        d = {}
        for g in range(2):
            loc = np.zeros(CH[g] * 128, dtype=np.int64)
            slo = np.full(CH[g] * 128, PADSLOT, dtype=np.float64)
            sel = gs == g
            ep, sl, bl = es[sel] - g * GRP, slot[sel], blks[sel]
            bstart = np.zeros(NB, dtype=np.int64)
            bstart[1:] = np.cumsum(np.bincount(bl, minlength=NB))[:-1]
            pos = off[g][bl] * 128 + (np.arange(len(ep)) - bstart[bl])
            loc[pos] = ep
            slo[pos] = sl
            d[f"idx{g}"] = wrap_idx(loc)
            d[f"dsb{g}"] = slo.reshape(-1, 128).T.astype(BF16)
        in_extra.append(d)

    plan = {"ct": ct, "CH": CH, "off": off,
            "nblk": node_blk, "nslot": node_slot}
    return dinv, in_extra, plan


def core_tables(c, xd, dinv, plan):
    """Per-core dst-side tables in the (possibly LPT-permuted) layout."""
    nblk, nslot = plan["nblk"], plan["nslot"]
    lo, hi = c * NPC, (c + 1) * NPC
    pos = nblk[lo:hi] * 128 + nslot[lo:hi]
    xdT = np.zeros((128, NPAD), dtype=BF16)
    xdT[:, pos] = xd[lo:hi].T
    dv = np.zeros(NPAD, dtype=np.float32)
    dv[pos] = dinv[lo:hi]
    dvc = dv.reshape(NB, 128).T.copy()
    wv = np.zeros(NPAD, dtype=np.float32)
    wv[pos] = 1.0 / dinv[lo:hi]
    winvc = wv.reshape(NB, 128).T.copy()
    return xdT, dvc, winvc


def _build(plan, bias_info):
    import concourse.bacc as bacc
    import concourse.tile as tile
    from concourse import mybir

    f32 = mybir.dt.float32
    f16 = mybir.dt.bfloat16
    i16 = mybir.dt.int16
    Alu = mybir.AluOpType
    Act = mybir.ActivationFunctionType

    ct, CH, off = plan["ct"], plan["CH"], plan["off"]
    has_bg, has_b1, has_b2, b3val = bias_info

    nc = bacc.Bacc("TRN2", target_bir_lowering=False, debug=False,
                   num_devices=NCORES,
                   dynamic_dma_scratch_size=DDS)

    def din(name, shape, dt=f32):
        return nc.dram_tensor(name, list(shape), dt, kind="ExternalInput")

    xdq_d = din("xdq", [N, D], f16)          # dinv*x, full table (bf16)
    xdT_d = din("xdT", [128, NPAD], f16)     # (dinv*x)[own].T
    dinvc_d = din("dinvc", [128, NB])
    winv_d = din("winvc", [128, NB])         # 1/dinv per (slot, block)
    wgT_d = din("wgT", [D, D], f16)          # Wg.T
    w1T_d = din("w1T", [128, 512])           # W1.T
    w2tp_d = din("w2tp", [128, 1024])        # W2.T row-blocks packed
    w3c_d = din("w3c", [128, 2])             # W3.T column chunks
    idx_d = [din(f"idx{g}", [128, int(CH[g]) * 8], i16) for g in range(2)]
    dsb_d = [din(f"dsb{g}", [128, int(CH[g])], f16) for g in range(2)]
    bgr_d = din("bgr", [1, 128]) if has_bg else None
    b1c_d = din("b1c", [128, 4]) if has_b1 else None
    b2c_d = din("b2c", [128, 2]) if has_b2 else None
    out_d = nc.dram_tensor("out", [1, 1], f32, kind="ExternalOutput")

    vb = nc.dram_tensor("vb", [1, 128], f32)
    vr = nc.dram_tensor("vr", [NCORES, 128], f32, addr_space="Shared")

    RG = [list(range(NCORES))]

    with tile.TileContext(nc) as tc:
        with (
            tc.tile_pool(name="const", bufs=1) as cpool,
            tc.tile_pool(name="seg", bufs=10) as segpool,
            tc.tile_pool(name="oh", bufs=10) as ohpool,
            tc.tile_pool(name="agg", bufs=3) as apool,
            tc.tile_pool(name="hb", bufs=3) as hpool,
            tc.tile_pool(name="mlp", bufs=1) as mpool,
            tc.tile_pool(name="psv", bufs=1, space="PSUM") as pvpool,
            tc.tile_pool(name="psS", bufs=3, space="PSUM") as pspool,
            tc.tile_pool(name="psH", bufs=2, space="PSUM") as phpool,
            tc.tile_pool(name="psT", bufs=1, space="PSUM") as ptpool,
        ):
            # ---- prefetch: gather-critical tables first ----
            # idx0 split so the first gather segment unblocks early
            idx_t = []
            it0 = cpool.tile([128, int(CH[0]) * 8], i16, tag="idx0")
            c_split = min(4 * SEGC * 8, int(CH[0]) * 8)
            nc.sync.dma_start(it0[:, 0:c_split], idx_d[0][:, 0:c_split])
            dsbh_t = []
            dh0 = cpool.tile([128, int(CH[0])], f16, tag="dsbh0")
            nc.sync.dma_start(dh0[:], dsb_d[0][:])
            nc.sync.dma_start(it0[:, c_split:int(CH[0]) * 8],
                              idx_d[0][:, c_split:int(CH[0]) * 8])
            idx_t.append(it0)
            dsbh_t.append(dh0)
            it1 = cpool.tile([128, int(CH[1]) * 8], i16, tag="idx1")
            nc.sync.dma_start(it1[:], idx_d[1][:])
            idx_t.append(it1)
            dh1 = cpool.tile([128, int(CH[1])], f16, tag="dsbh1")
            nc.sync.dma_start(dh1[:], dsb_d[1][:])
            dsbh_t.append(dh1)
            dinv_t = cpool.tile([128, NB], f32)
            nc.sync.dma_start(dinv_t[:], dinvc_d[:])
            winv_t = cpool.tile([128, NB], f32, tag="winv")
            nc.sync.dma_start(winv_t[:], winv_d[:])
            wgT_t = cpool.tile([128, 128], f16, tag="wgT")
            nc.sync.dma_start(wgT_t[:], wgT_d[:])
            xdT_t = cpool.tile([128, NPAD], f16, tag="xdT")
            nc.sync.dma_start(xdT_t[:], xdT_d[:])

            # on-chip constants (no DMA): iota row, identities, ones
            iota_t = cpool.tile([128, 128], f16)
            nc.gpsimd.iota(iota_t[:], [[1, 128]], channel_multiplier=0,
                           allow_small_or_imprecise_dtypes=True)
            pcol_t = cpool.tile([128, 1], f32, tag="pcol")
            nc.gpsimd.iota(pcol_t[:], [[0, 1]], channel_multiplier=1,
                           allow_small_or_imprecise_dtypes=True)
            ones_t = cpool.tile([128, 1], f16)
            nc.vector.memset(ones_t[:], 1.0)
            # dsb tables cast to f32 (is_equal scalar operand must be f32)
            dsb_t = []
            for g in range(2):
                dt_ = cpool.tile([128, int(CH[g])], f32, tag=f"dsb{g}")
                nc.vector.tensor_copy(dt_[:], dsbh_t[g][:])
                dsb_t.append(dt_)
            idh_t = cpool.tile([128, 128], f16)
            nc.vector.tensor_scalar(idh_t[:], iota_t[:], pcol_t[:],
                                    None, Alu.is_equal)
            idf_t = cpool.tile([128, 128], f32)
            nc.vector.tensor_scalar(idf_t[:], iota_t[:], pcol_t[:],
                                    None, Alu.is_equal)

            if has_bg:
                bgr_t = cpool.tile([1, 128], f32, tag="bgr")
                nc.sync.dma_start(bgr_t[:], bgr_d[:])
                # materialize bg as [128,128] via ones-outer-product
                ones_f = cpool.tile([128, 1], f32, tag="onesf")
                nc.vector.memset(ones_f[:], 1.0)
                psb = ptpool.tile([128, 128], f32, tag="pst")
                nc.tensor.matmul(psb[:], ones_f[:], bgr_t[:],
                                 start=True, stop=True)
                bg_t = cpool.tile([128, 128], f32, tag="bgt")
                nc.vector.tensor_copy(bg_t[:], psb[:])

            # ---- main: gather + one-hot segment-sum + W apply ----
            ytab = [xdq_d[0:GRP, :], xdq_d[GRP:N, :]]
            nseg = [int(np.ceil(CH[g] / SEGC)) for g in range(2)]
            seg_tiles = [[None] * nseg[g] for g in range(2)]
            oh_tiles = [[None] * nseg[g] for g in range(2)]
            seg_ptr = [0, 0]

            def ensure_seg(g, s):
                while seg_ptr[g] <= s:
                    si = seg_ptr[g]
                    ncols = min(SEGC, int(CH[g]) - si * SEGC)
                    tl = segpool.tile([128, ncols, 128], f16, tag="seg")
                    nidx = ncols * 128
                    nc.gpsimd.dma_gather(
                        tl[:], ytab[g], idx_t[g][:, si * (SEGC * 8):
                                                 si * (SEGC * 8) + ncols * 8],
                        num_idxs=nidx, num_idxs_reg=nidx,
                        elem_size=128, elem_step=128)
                    seg_tiles[g][si] = tl
                    oh = ohpool.tile([128, ncols, 128], f16, tag="oh")
                    for cc in range(ncols):
                        nc.vector.tensor_scalar(
                            oh[:, cc, :], iota_t[:],
                            dsb_t[g][:, si * SEGC + cc: si * SEGC + cc + 1],
                            None, Alu.is_equal)
                    oh_tiles[g][si] = oh
                    seg_ptr[g] += 1

            psv = pvpool.tile([1, 128], f32, tag="psv")
            first_v = [True]
            nblk = (0 if SKIP_MAIN else
                    (NB if DEBUG_BLOCKS is None else DEBUG_BLOCKS))
            for b in range(nblk):
                psS = pspool.tile([128, 128], f32, tag="psS")
                tot = int(ct[0][b] + ct[1][b])
                k = 0
                for g in range(2):
                    for j in range(int(ct[g][b])):
                        ci = int(off[g][b]) + j
                        s, col = divmod(ci, SEGC)
                        ensure_seg(g, s)
                        nc.tensor.matmul(psS[:], seg_tiles[g][s][:, col, :],
                                         oh_tiles[g][s][:, col, :],
                                         start=(k == 0), stop=(k == tot - 1))
                        k += 1
                # aggT: [fin, slot] PSUM -> SBUF bf16 (ACT engine)
                aggT = apool.tile([128, 128], f16, tag="agg")
                if tot == 0:
                    nc.vector.memset(aggT[:], 0.0)
                else:
                    nc.scalar.activation(aggT[:], psS[:], Act.Copy)
                # W apply + self-loop term
                psH = phpool.tile([128, 128], f32, tag="psH")
                nc.tensor.matmul(psH[:], aggT[:], wgT_t[:],
                                 start=True, stop=False)
                nc.tensor.matmul(psH[:], xdT_t[:, b * 128:(b + 1) * 128],
                                 wgT_t[:], start=False, stop=True)
                hb = hpool.tile([128, 128], f16)
                if has_bg:
                    tmp = hpool.tile([128, 128], f32, tag="tmp")
                    nc.vector.tensor_scalar(tmp[:], psH[:],
                                            dinv_t[:, b:b + 1], None, Alu.mult)
                    nc.vector.tensor_tensor(tmp[:], tmp[:], bg_t[:], Alu.add)
                    nc.scalar.activation(hb[:], tmp[:], Act.Relu)
                    if b == NB - 1:
                        nc.vector.memset(hb[NPC - (NB - 1) * 128:128, :], 0.0)
                else:
                    nc.scalar.activation(hb[:], psH[:], Act.Relu,
                                         scale=dinv_t[:, b:b + 1])
                nc.tensor.matmul(psv[:], ones_t[:], hb[:],
                                 start=first_v[0], stop=False,
                                 skip_group_check=True)
                first_v[0] = False
                # residual colsum via xdT: x rows = (1/dinv) * xd rows
                pstx = ptpool.tile([128, 128], f16, tag="pst")
                nc.tensor.transpose(pstx[:], xdT_t[:, b * 128:(b + 1) * 128],
                                    idh_t[:])
                xdr = hpool.tile([128, 128], f32, tag="xdr")
                nc.scalar.activation(xdr[:], pstx[:], Act.Copy)
                nc.tensor.matmul(psv[:], winv_t[:, b:b + 1], xdr[:],
                                 start=False, stop=(b == nblk - 1),
                                 skip_group_check=True)

            # late-use loads (overlap the tail of the gather stream)
            w1T_t = mpool.tile([128, 512], f32, tag="w1T")
            nc.sync.dma_start(w1T_t[:], w1T_d[:])
            w2tp_t = mpool.tile([128, 1024], f32, tag="w2tp")
            nc.sync.dma_start(w2tp_t[:], w2tp_d[:])
            w3c_t = mpool.tile([128, 2], f32, tag="w3c")
            nc.sync.dma_start(w3c_t[:], w3c_d[:])
            if has_b1:
                b1c_t = mpool.tile([128, 4], f32, tag="b1c")
                nc.sync.dma_start(b1c_t[:], b1c_d[:])
            if has_b2:
                b2c_t = mpool.tile([128, 2], f32, tag="b2c")
                nc.sync.dma_start(b2c_t[:], b2c_d[:])

            # ---- v exchange: AllGather partial rows, sum on-chip ----
            vrow = mpool.tile([1, 128], f32, tag="vrow")
            if nblk == 0:
                nc.vector.memset(vrow[:], 0.0)
            else:
                nc.scalar.copy(vrow[:], psv[:])
            nc.sync.dma_start(vb[:], vrow[:])
            nc.gpsimd.collective_compute(
                "AllGather", Alu.bypass, replica_groups=RG,
                ins=[vb[:]], outs=[vr[:]])
            vgt = mpool.tile([NCORES, 128], f32, tag="vgt")
            nc.sync.dma_start(vgt[:], vr[:])
            ones8 = mpool.tile([NCORES, 1], f32, tag="ones8")
            nc.vector.memset(ones8[:], 1.0)
            psv2 = pvpool.tile([1, 128], f32, tag="psv")
            nc.tensor.matmul(psv2[:], ones8[:], vgt[:], start=True, stop=True)
            vfull = mpool.tile([1, 128], f32, tag="vfull")
            nc.scalar.copy(vfull[:], psv2[:])

            # ---- MLP head ----
            if SKIP_MLP:
                nc.sync.dma_start(out_d[:], vfull[0:1, 0:1])
            else:
                pst = ptpool.tile([128, 1], f32, tag="pst")
                nc.tensor.transpose(pst[:], vfull[:], idf_t[0:1, 0:1])
                vcol = mpool.tile([128, 1], f32, tag="vcol")
                nc.vector.tensor_copy(vcol[:], pst[:])

                a1c = []
                for m in range(4):
                    ps1 = ptpool.tile([128, 1], f32, tag="pst")
                    nc.tensor.matmul(ps1[:], w1T_t[:, m * 128:(m + 1) * 128],
                                     vcol[:], start=True, stop=True)
                    a1 = mpool.tile([128, 1], f32, tag=f"a1c{m}")
                    if has_b1:
                        nc.scalar.activation(a1[:], ps1[:], Act.Relu,
                                             bias=b1c_t[:, m:m + 1])
                    else:
                        nc.scalar.activation(a1[:], ps1[:], Act.Relu)
                    a1c.append(a1)

                a2c = []
                for m in range(2):
                    ps2 = ptpool.tile([128, 1], f32, tag="pst")
                    for kk in range(4):
                        nc.tensor.matmul(
                            ps2[:],
                            w2tp_t[:, kk * 256 + m * 128:
                                   kk * 256 + (m + 1) * 128],
                            a1c[kk][:], start=(kk == 0), stop=(kk == 3))
                    a2 = mpool.tile([128, 1], f32, tag=f"a2c{m}")
                    if has_b2:
                        nc.scalar.activation(a2[:], ps2[:], Act.Relu,
                                             bias=b2c_t[:, m:m + 1])
                    else:
                        nc.scalar.activation(a2[:], ps2[:], Act.Relu)
                    a2c.append(a2)

                ps3 = ptpool.tile([1, 1], f32, tag="ps3")
                for kk in range(2):
                    nc.tensor.matmul(ps3[:], w3c_t[:, kk:kk + 1], a2c[kk][:],
                                     start=(kk == 0), stop=(kk == 1))
                ot = mpool.tile([1, 1], f32, tag="ot")
                nc.scalar.activation(ot[:], ps3[:], Act.Copy,
                                     bias=float(b3val))
                nc.sync.dma_start(out_d[:], ot[:])

    nc.compile()
    return nc


TRACE = False
LAST_EXEC_NS = None
LAST_RESULT = None


def kernel(**inputs):
    from concourse.bass_utils import run_bass_kernel_spmd

    x = np.asarray(inputs["x"], dtype=np.float32)
    Wg = np.asarray(inputs["Wg"], dtype=np.float32)
    bg = np.asarray(inputs["bg"], dtype=np.float32)
    W1 = np.asarray(inputs["W1"], dtype=np.float32)
    b1 = np.asarray(inputs["b1"], dtype=np.float32)
    W2 = np.asarray(inputs["W2"], dtype=np.float32)
    b2 = np.asarray(inputs["b2"], dtype=np.float32)
    W3 = np.asarray(inputs["W3"], dtype=np.float32)
    b3 = np.asarray(inputs["b3"], dtype=np.float32)

    bias_info = (bool(bg.any()), bool(b1.any()), bool(b2.any()),
                 float(b3.reshape(-1)[0]))
    # LPT layout leaves empty slots scattered across blocks, which the
    # bg!=0 path cannot mask; fall back to the sequential layout there.
    dinv, in_extra, plan = _prep(inputs["edge_index"],
                                 use_lpt=not bias_info[0])
    nc = _build(plan, bias_info)

    xd = (dinv[:, None] * x).astype(BF16)           # [N, D] table
    wgT = Wg.T.astype(BF16).copy()
    w1T = W1.T.astype(np.float32).copy()            # [128, 512]
    w2tp = np.concatenate([W2.T[k * 128:(k + 1) * 128] for k in range(4)],
                          axis=1).astype(np.float32).copy()  # [128, 1024]
    w3c = W3.reshape(2, 128).T.astype(np.float32).copy()     # [128, 2]

    in_maps = []
    for c in range(NCORES):
        xdT, dvc, winvc = core_tables(c, xd, dinv, plan)
        m = {"xdq": xd, "xdT": xdT, "winvc": winvc, "dinvc": dvc,
             "wgT": wgT, "w1T": w1T, "w2tp": w2tp, "w3c": w3c,
             "idx0": in_extra[c]["idx0"], "idx1": in_extra[c]["idx1"],
             "dsb0": in_extra[c]["dsb0"], "dsb1": in_extra[c]["dsb1"]}
        if bias_info[0]:
            m["bgr"] = bg.reshape(1, 128)
        if bias_info[1]:
            m["b1c"] = b1.reshape(4, 128).T.astype(np.float32).copy()
        if bias_info[2]:
            m["b2c"] = b2.reshape(2, 128).T.astype(np.float32).copy()
        in_maps.append(m)

    res = run_bass_kernel_spmd(nc, in_maps, list(range(NCORES)), trace=TRACE)
    global LAST_EXEC_NS, LAST_RESULT
    LAST_EXEC_NS = res.exec_time_ns
    LAST_RESULT = res
    return res.results[0]["out"].reshape(1).astype(np.float32)


# revision 41
# speedup vs baseline: 1.0506x; 1.0065x over previous
"""GCN critic network kernel for 8 TRN2 NeuronCores.

Strategy (dst-shard, no message-table collective):
  The GCNConv linear commutes with the segment sum:
      out[d] = dinv_d * (sum_{s in N(d)} dinv_s * x[s]) @ Wg.T + bg
  so each core gathers pre-scaled raw rows xd = dinv*x (host-prepped bf16
  table in DRAM, a kernel input) for its own dst shard's edges and applies
  Wg once per 128-dst block after aggregation. This removes the y-table
  AllGather of the previous design entirely; the only collective left is
  the tiny [1,128] AllReduce of the pooled vector.

  - dst nodes sharded 6250/core (padded 6272 = 49 blocks of 128).
  - Edge messages: dma_gather of 256B bf16 rows (int16 indices, node table
    split in two <=32768-row groups), segment-summed per dst block via
    one-hot matmul accumulation in PSUM:  psST[fin, slot] += seg.T @ oh.
  - One-hot built per chunk with a single DVE tensor_scalar is_equal
    (iota row constant vs per-partition dst-slot scalar) - hits the
    packed-2-byte DVE fast path.
  - Self-loops folded in as one extra matmul per block from the resident
    xdT slice (no gathered self rows).
  - h = relu(dinv * (aggT.T @ WgT)); v = ones@h blocks + colsum(x_own);
    AllReduce v; tiny MLP head with host-pretransposed weights.
"""

import os
import numpy as np
import ml_dtypes

BF16 = ml_dtypes.bfloat16
N = 50000
E = 800000
D = 128
NCORES = 8
NPC = 6250          # dst nodes per core
NPAD = 6272         # padded (49 * 128)
NB = NPAD // 128    # dst blocks per core
GRP = 32768         # int16 index-group boundary (table-row space)
SEGC = int(os.environ.get("KB_SEGC", "8"))   # chunks per gather call
DDS = int(os.environ.get("KB_DDS", "65536"))
PADSLOT = 300.0     # dst-slot sentinel for padding rows (one-hot miss)

DEBUG_BLOCKS = (int(os.environ["KB_DEBUG_BLOCKS"])
                if "KB_DEBUG_BLOCKS" in os.environ else None)
SKIP_MLP = bool(os.environ.get("KB_SKIP_MLP"))
SKIP_MAIN = bool(os.environ.get("KB_SKIP_MAIN"))


def _lpt_assign(src, dst):
    """Per-core 2D-LPT node->(block, slot) map balancing per-(group, block)
    edge counts across cores; shrinks the uniform chunk-count padding."""
    c0 = np.bincount(dst[src < GRP], minlength=N)
    c1 = np.bincount(dst[src >= GRP], minlength=N)
    node_blk = np.empty(N, np.int64)
    node_slot = np.empty(N, np.int64)
    for c in range(NCORES):
        nodes = np.arange(c * NPC, (c + 1) * NPC)
        a0, a1 = c0[nodes].astype(np.float64), c1[nodes].astype(np.float64)
        order = np.argsort(-(a0 + a1), kind="stable")
        T0 = max(a0.sum() / NB, 1.0)
        T1 = max(a1.sum() / NB, 1.0)
        w0 = np.zeros(NB)
        w1 = np.zeros(NB)
        ns = np.zeros(NB, np.int64)
        for j in order:
            pen = np.where(ns < 128,
                           np.maximum((w0 + a0[j]) / T0, (w1 + a1[j]) / T1),
                           np.inf)
            b = int(np.argmin(pen))
            node_blk[nodes[j]] = b
            node_slot[nodes[j]] = ns[b]
            w0[b] += a0[j]
            w1[b] += a1[j]
            ns[b] += 1
    return node_blk, node_slot


def _prep(edge_index, use_lpt=True):
    """Host-side graph prep: per-core chunked edge layout + uniform plan."""
    src = np.asarray(edge_index[0]).astype(np.int64)
    dst = np.asarray(edge_index[1]).astype(np.int64)

    deg = np.bincount(dst, minlength=N).astype(np.float64) + 1.0
    dinv = (1.0 / np.sqrt(deg)).astype(np.float32)

    if use_lpt and not os.environ.get("KB_NOLPT"):
        node_blk, node_slot = _lpt_assign(src, dst)
    else:
        local = np.arange(N) % NPC
        node_blk = local >> 7
        node_slot = local & 127

    per_core = []
    cnt = np.zeros((NCORES, 2, NB), dtype=np.int64)
    for c in range(NCORES):
        lo, hi = c * NPC, (c + 1) * NPC
        m = (dst >= lo) & (dst < hi)
        es = src[m]
        ds = dst[m]
        g = (es >= GRP).astype(np.int64)
        blk = node_blk[ds]
        slot = node_slot[ds]
        # sort by (group, block, src) - src order improves HBM locality
        order = np.lexsort((es, blk, g))
        es, slot, g, blk = es[order], slot[order], g[order], blk[order]
        np.add.at(cnt[c], (g, blk), 1)
        per_core.append((es, slot, g, blk))

    # uniform chunk counts per (group, block) across cores (SPMD program)
    ct = np.ceil(cnt.max(axis=0) / 128.0).astype(np.int64)  # [2, NB]
    CH = ct.sum(axis=1)                                     # chunks per group
    off = np.zeros((2, NB), dtype=np.int64)
    off[:, 1:] = np.cumsum(ct, axis=1)[:, :-1]

    def wrap_idx(loc):
        nch = len(loc) // 128
        cols = []
        for s0 in range(0, nch, SEGC):
            seg = loc[s0 * 128: min(nch, s0 + SEGC) * 128]
            a = seg.reshape(-1, 16).T.astype(np.int16)        # [16, cols_s]
            cols.append(np.tile(a, (8, 1)))                   # [128, cols_s]
        return np.concatenate(cols, axis=1)

    in_extra = []
    for c in range(NCORES):
        es, slot, gs, blks = per_core[c]
        d = {}
        for g in range(2):
            loc = np.zeros(CH[g] * 128, dtype=np.int64)
            slo = np.full(CH[g] * 128, PADSLOT, dtype=np.float64)
            sel = gs == g
            ep, sl, bl = es[sel] - g * GRP, slot[sel], blks[sel]
            bstart = np.zeros(NB, dtype=np.int64)
            bstart[1:] = np.cumsum(np.bincount(bl, minlength=NB))[:-1]
            pos = off[g][bl] * 128 + (np.arange(len(ep)) - bstart[bl])
            loc[pos] = ep
            slo[pos] = sl
            d[f"idx{g}"] = wrap_idx(loc)
            d[f"dsb{g}"] = slo.reshape(-1, 128).T.astype(BF16)
        in_extra.append(d)

    plan = {"ct": ct, "CH": CH, "off": off,
            "nblk": node_blk, "nslot": node_slot}
    return dinv, in_extra, plan


def core_tables(c, xd, dinv, plan):
    """Per-core dst-side tables in the (possibly LPT-permuted) layout."""
    nblk, nslot = plan["nblk"], plan["nslot"]
    lo, hi = c * NPC, (c + 1) * NPC
    pos = nblk[lo:hi] * 128 + nslot[lo:hi]
    xdT = np.zeros((128, NPAD), dtype=BF16)
    xdT[:, pos] = xd[lo:hi].T
    dv = np.zeros(NPAD, dtype=np.float32)
    dv[pos] = dinv[lo:hi]
    dvc = dv.reshape(NB, 128).T.copy()
    wv = np.zeros(NPAD, dtype=np.float32)
    wv[pos] = 1.0 / dinv[lo:hi]
    winvc = wv.reshape(NB, 128).T.copy()
    return xdT, dvc, winvc


def _build(plan, bias_info):
    import concourse.bacc as bacc
    import concourse.tile as tile
    from concourse import mybir

    f32 = mybir.dt.float32
    f16 = mybir.dt.bfloat16
    i16 = mybir.dt.int16
    Alu = mybir.AluOpType
    Act = mybir.ActivationFunctionType

    ct, CH, off = plan["ct"], plan["CH"], plan["off"]
    has_bg, has_b1, has_b2, b3val = bias_info

    nc = bacc.Bacc("TRN2", target_bir_lowering=False, debug=False,
                   num_devices=NCORES,
                   dynamic_dma_scratch_size=DDS)

    def din(name, shape, dt=f32):
        return nc.dram_tensor(name, list(shape), dt, kind="ExternalInput")

    xdq_d = din("xdq", [N, D], f16)          # dinv*x, full table (bf16)
    xdT_d = din("xdT", [128, NPAD], f16)     # (dinv*x)[own].T
    dinvc_d = din("dinvc", [128, NB])
    winv_d = din("winvc", [128, NB])         # 1/dinv per (slot, block)
    wgT_d = din("wgT", [D, D], f16)          # Wg.T
    w1T_d = din("w1T", [128, 512])           # W1.T
    w2tp_d = din("w2tp", [128, 1024])        # W2.T row-blocks packed
    w3c_d = din("w3c", [128, 2])             # W3.T column chunks
    idx_d = [din(f"idx{g}", [128, int(CH[g]) * 8], i16) for g in range(2)]
    dsb_d = [din(f"dsb{g}", [128, int(CH[g])], f16) for g in range(2)]
    bgr_d = din("bgr", [1, 128]) if has_bg else None
    b1c_d = din("b1c", [128, 4]) if has_b1 else None
    b2c_d = din("b2c", [128, 2]) if has_b2 else None
    out_d = nc.dram_tensor("out", [1, 1], f32, kind="ExternalOutput")

    vb = nc.dram_tensor("vb", [1, 128], f32)
    vr = nc.dram_tensor("vr", [NCORES, 128], f32, addr_space="Shared")

    RG = [list(range(NCORES))]

    with tile.TileContext(nc) as tc:
        with (
            tc.tile_pool(name="const", bufs=1) as cpool,
            tc.tile_pool(name="seg", bufs=10) as segpool,
            tc.tile_pool(name="oh", bufs=10) as ohpool,
            tc.tile_pool(name="agg", bufs=3) as apool,
            tc.tile_pool(name="hb", bufs=3) as hpool,
            tc.tile_pool(name="mlp", bufs=1) as mpool,
            tc.tile_pool(name="psv", bufs=1, space="PSUM") as pvpool,
            tc.tile_pool(name="psS", bufs=3, space="PSUM") as pspool,
            tc.tile_pool(name="psH", bufs=2, space="PSUM") as phpool,
            tc.tile_pool(name="psT", bufs=1, space="PSUM") as ptpool,
        ):
            # ---- prefetch: gather-critical tables first ----
            # idx0 split so the first gather segment unblocks early
            idx_t = []
            it0 = cpool.tile([128, int(CH[0]) * 8], i16, tag="idx0")
            c_split = min(4 * SEGC * 8, int(CH[0]) * 8)
            nc.sync.dma_start(it0[:, 0:c_split], idx_d[0][:, 0:c_split])
            dsbh_t = []
            dh0 = cpool.tile([128, int(CH[0])], f16, tag="dsbh0")
            nc.sync.dma_start(dh0[:], dsb_d[0][:])
            nc.sync.dma_start(it0[:, c_split:int(CH[0]) * 8],
                              idx_d[0][:, c_split:int(CH[0]) * 8])
            idx_t.append(it0)
            dsbh_t.append(dh0)
            it1 = cpool.tile([128, int(CH[1]) * 8], i16, tag="idx1")
            nc.sync.dma_start(it1[:], idx_d[1][:])
            idx_t.append(it1)
            dh1 = cpool.tile([128, int(CH[1])], f16, tag="dsbh1")
            nc.sync.dma_start(dh1[:], dsb_d[1][:])
            dsbh_t.append(dh1)
            dinv_t = cpool.tile([128, NB], f32)
            nc.sync.dma_start(dinv_t[:], dinvc_d[:])
            winv_t = cpool.tile([128, NB], f32, tag="winv")
            nc.sync.dma_start(winv_t[:], winv_d[:])
            wgT_t = cpool.tile([128, 128], f16, tag="wgT")
            nc.sync.dma_start(wgT_t[:], wgT_d[:])
            xdT_t = cpool.tile([128, NPAD], f16, tag="xdT")
            nc.sync.dma_start(xdT_t[:], xdT_d[:])

            # on-chip constants (no DMA): iota row, identities, ones
            iota_t = cpool.tile([128, 128], f16)
            nc.gpsimd.iota(iota_t[:], [[1, 128]], channel_multiplier=0,
                           allow_small_or_imprecise_dtypes=True)
            pcol_t = cpool.tile([128, 1], f32, tag="pcol")
            nc.gpsimd.iota(pcol_t[:], [[0, 1]], channel_multiplier=1,
                           allow_small_or_imprecise_dtypes=True)
            ones_t = cpool.tile([128, 1], f16)
            nc.vector.memset(ones_t[:], 1.0)
            # dsb tables cast to f32 (is_equal scalar operand must be f32)
            dsb_t = []
            for g in range(2):
                dt_ = cpool.tile([128, int(CH[g])], f32, tag=f"dsb{g}")
                nc.vector.tensor_copy(dt_[:], dsbh_t[g][:])
                dsb_t.append(dt_)
            idh_t = cpool.tile([128, 128], f16)
            nc.vector.tensor_scalar(idh_t[:], iota_t[:], pcol_t[:],
                                    None, Alu.is_equal)
            idf_t = cpool.tile([128, 128], f32)
            nc.vector.tensor_scalar(idf_t[:], iota_t[:], pcol_t[:],
                                    None, Alu.is_equal)

            if has_bg:
                bgr_t = cpool.tile([1, 128], f32, tag="bgr")
                nc.sync.dma_start(bgr_t[:], bgr_d[:])
                # materialize bg as [128,128] via ones-outer-product
                ones_f = cpool.tile([128, 1], f32, tag="onesf")
                nc.vector.memset(ones_f[:], 1.0)
                psb = ptpool.tile([128, 128], f32, tag="pst")
                nc.tensor.matmul(psb[:], ones_f[:], bgr_t[:],
                                 start=True, stop=True)
                bg_t = cpool.tile([128, 128], f32, tag="bgt")
                nc.vector.tensor_copy(bg_t[:], psb[:])

            # ---- main: gather + one-hot segment-sum + W apply ----
            ytab = [xdq_d[0:GRP, :], xdq_d[GRP:N, :]]
            nseg = [int(np.ceil(CH[g] / SEGC)) for g in range(2)]
            seg_tiles = [[None] * nseg[g] for g in range(2)]
            oh_tiles = [[None] * nseg[g] for g in range(2)]
            seg_ptr = [0, 0]

            def ensure_seg(g, s):
                while seg_ptr[g] <= s:
                    si = seg_ptr[g]
                    ncols = min(SEGC, int(CH[g]) - si * SEGC)
                    tl = segpool.tile([128, ncols, 128], f16, tag="seg")
                    nidx = ncols * 128
                    nc.gpsimd.dma_gather(
                        tl[:], ytab[g], idx_t[g][:, si * (SEGC * 8):
                                                 si * (SEGC * 8) + ncols * 8],
                        num_idxs=nidx, num_idxs_reg=nidx,
                        elem_size=128, elem_step=128)
                    seg_tiles[g][si] = tl
                    oh = ohpool.tile([128, ncols, 128], f16, tag="oh")
                    for cc in range(ncols):
                        nc.vector.tensor_scalar(
                            oh[:, cc, :], iota_t[:],
                            dsb_t[g][:, si * SEGC + cc: si * SEGC + cc + 1],
                            None, Alu.is_equal)
                    oh_tiles[g][si] = oh
                    seg_ptr[g] += 1

            psv = pvpool.tile([1, 128], f32, tag="psv")
            first_v = [True]
            nblk = (0 if SKIP_MAIN else
                    (NB if DEBUG_BLOCKS is None else DEBUG_BLOCKS))
            for b in range(nblk):
                # residual colsum via xdT: x rows = (1/dinv) * xd rows.
                # Issued at iteration top so block b's chain overlaps its
                # own gather/matmul stream; psv group closes on final vacc.
                pstx = ptpool.tile([128, 128], f16, tag="pst")
                nc.tensor.transpose(pstx[:], xdT_t[:, b * 128:(b + 1) * 128],
                                    idh_t[:])
                xdr = hpool.tile([128, 128], f32, tag="xdr")
                nc.scalar.activation(xdr[:], pstx[:], Act.Copy)
                nc.tensor.matmul(psv[:], winv_t[:, b:b + 1], xdr[:],
                                 start=first_v[0], stop=False,
                                 skip_group_check=True)
                first_v[0] = False
                psS = pspool.tile([128, 128], f32, tag="psS")
                tot = int(ct[0][b] + ct[1][b])
                k = 0
                for g in range(2):
                    for j in range(int(ct[g][b])):
                        ci = int(off[g][b]) + j
                        s, col = divmod(ci, SEGC)
                        ensure_seg(g, s)
                        nc.tensor.matmul(psS[:], seg_tiles[g][s][:, col, :],
                                         oh_tiles[g][s][:, col, :],
                                         start=(k == 0), stop=(k == tot - 1))
                        k += 1
                # aggT: [fin, slot] PSUM -> SBUF bf16 (ACT engine)
                aggT = apool.tile([128, 128], f16, tag="agg")
                if tot == 0:
                    nc.vector.memset(aggT[:], 0.0)
                else:
                    nc.scalar.activation(aggT[:], psS[:], Act.Copy)
                # W apply + self-loop term
                psH = phpool.tile([128, 128], f32, tag="psH")
                nc.tensor.matmul(psH[:], aggT[:], wgT_t[:],
                                 start=True, stop=False)
                nc.tensor.matmul(psH[:], xdT_t[:, b * 128:(b + 1) * 128],
                                 wgT_t[:], start=False, stop=True)
                hb = hpool.tile([128, 128], f16)
                if has_bg:
                    tmp = hpool.tile([128, 128], f32, tag="tmp")
                    nc.vector.tensor_scalar(tmp[:], psH[:],
                                            dinv_t[:, b:b + 1], None, Alu.mult)
                    nc.vector.tensor_tensor(tmp[:], tmp[:], bg_t[:], Alu.add)
                    nc.scalar.activation(hb[:], tmp[:], Act.Relu)
                    if b == NB - 1:
                        nc.vector.memset(hb[NPC - (NB - 1) * 128:128, :], 0.0)
                else:
                    nc.scalar.activation(hb[:], psH[:], Act.Relu,
                                         scale=dinv_t[:, b:b + 1])
                nc.tensor.matmul(psv[:], ones_t[:], hb[:],
                                 start=False, stop=(b == nblk - 1),
                                 skip_group_check=True)

            # late-use loads (overlap the tail of the gather stream)
            w1T_t = mpool.tile([128, 512], f32, tag="w1T")
            nc.sync.dma_start(w1T_t[:], w1T_d[:])
            w2tp_t = mpool.tile([128, 1024], f32, tag="w2tp")
            nc.sync.dma_start(w2tp_t[:], w2tp_d[:])
            w3c_t = mpool.tile([128, 2], f32, tag="w3c")
            nc.sync.dma_start(w3c_t[:], w3c_d[:])
            if has_b1:
                b1c_t = mpool.tile([128, 4], f32, tag="b1c")
                nc.sync.dma_start(b1c_t[:], b1c_d[:])
            if has_b2:
                b2c_t = mpool.tile([128, 2], f32, tag="b2c")
                nc.sync.dma_start(b2c_t[:], b2c_d[:])

            # ---- v exchange: AllGather partial rows, sum on-chip ----
            vrow = mpool.tile([1, 128], f32, tag="vrow")
            if nblk == 0:
                nc.vector.memset(vrow[:], 0.0)
            else:
                nc.scalar.copy(vrow[:], psv[:])
            nc.sync.dma_start(vb[:], vrow[:])
            nc.gpsimd.collective_compute(
                "AllGather", Alu.bypass, replica_groups=RG,
                ins=[vb[:]], outs=[vr[:]])
            vgt = mpool.tile([NCORES, 128], f32, tag="vgt")
            nc.sync.dma_start(vgt[:], vr[:])
            ones8 = mpool.tile([NCORES, 1], f32, tag="ones8")
            nc.vector.memset(ones8[:], 1.0)
            # ---- MLP head ----
            if SKIP_MLP:
                psv2 = pvpool.tile([1, 128], f32, tag="psv")
                nc.tensor.matmul(psv2[:], ones8[:], vgt[:],
                                 start=True, stop=True)
                vfull = mpool.tile([1, 128], f32, tag="vfull")
                nc.scalar.copy(vfull[:], psv2[:])
                nc.sync.dma_start(out_d[:], vfull[0:1, 0:1])
            else:
                psv3 = ptpool.tile([128, 1], f32, tag="pst")
                nc.tensor.matmul(psv3[:], vgt[:], ones8[:],
                                 start=True, stop=True)
                vcol = mpool.tile([128, 1], f32, tag="vcol")
                nc.vector.tensor_copy(vcol[:], psv3[:])

                a1c = []
                for m in range(4):
                    ps1 = ptpool.tile([128, 1], f32, tag="pst")
                    nc.tensor.matmul(ps1[:], w1T_t[:, m * 128:(m + 1) * 128],
                                     vcol[:], start=True, stop=True)
                    a1 = mpool.tile([128, 1], f32, tag=f"a1c{m}")
                    if has_b1:
                        nc.scalar.activation(a1[:], ps1[:], Act.Relu,
                                             bias=b1c_t[:, m:m + 1])
                    else:
                        nc.scalar.activation(a1[:], ps1[:], Act.Relu)
                    a1c.append(a1)

                a2c = []
                for m in range(2):
                    ps2 = ptpool.tile([128, 1], f32, tag="pst")
                    for kk in range(4):
                        nc.tensor.matmul(
                            ps2[:],
                            w2tp_t[:, kk * 256 + m * 128:
                                   kk * 256 + (m + 1) * 128],
                            a1c[kk][:], start=(kk == 0), stop=(kk == 3))
                    a2 = mpool.tile([128, 1], f32, tag=f"a2c{m}")
                    if has_b2:
                        nc.scalar.activation(a2[:], ps2[:], Act.Relu,
                                             bias=b2c_t[:, m:m + 1])
                    else:
                        nc.scalar.activation(a2[:], ps2[:], Act.Relu)
                    a2c.append(a2)

                ps3 = ptpool.tile([1, 1], f32, tag="ps3")
                for kk in range(2):
                    nc.tensor.matmul(ps3[:], w3c_t[:, kk:kk + 1], a2c[kk][:],
                                     start=(kk == 0), stop=(kk == 1))
                ot = mpool.tile([1, 1], f32, tag="ot")
                nc.scalar.activation(ot[:], ps3[:], Act.Copy,
                                     bias=float(b3val))
                nc.sync.dma_start(out_d[:], ot[:])

    nc.compile()
    return nc


TRACE = False
LAST_EXEC_NS = None
LAST_RESULT = None


def kernel(**inputs):
    from concourse.bass_utils import run_bass_kernel_spmd

    x = np.asarray(inputs["x"], dtype=np.float32)
    Wg = np.asarray(inputs["Wg"], dtype=np.float32)
    bg = np.asarray(inputs["bg"], dtype=np.float32)
    W1 = np.asarray(inputs["W1"], dtype=np.float32)
    b1 = np.asarray(inputs["b1"], dtype=np.float32)
    W2 = np.asarray(inputs["W2"], dtype=np.float32)
    b2 = np.asarray(inputs["b2"], dtype=np.float32)
    W3 = np.asarray(inputs["W3"], dtype=np.float32)
    b3 = np.asarray(inputs["b3"], dtype=np.float32)

    bias_info = (bool(bg.any()), bool(b1.any()), bool(b2.any()),
                 float(b3.reshape(-1)[0]))
    # LPT layout leaves empty slots scattered across blocks, which the
    # bg!=0 path cannot mask; fall back to the sequential layout there.
    dinv, in_extra, plan = _prep(inputs["edge_index"],
                                 use_lpt=not bias_info[0])
    nc = _build(plan, bias_info)

    xd = (dinv[:, None] * x).astype(BF16)           # [N, D] table
    wgT = Wg.T.astype(BF16).copy()
    w1T = W1.T.astype(np.float32).copy()            # [128, 512]
    w2tp = np.concatenate([W2.T[k * 128:(k + 1) * 128] for k in range(4)],
                          axis=1).astype(np.float32).copy()  # [128, 1024]
    w3c = W3.reshape(2, 128).T.astype(np.float32).copy()     # [128, 2]

    in_maps = []
    for c in range(NCORES):
        xdT, dvc, winvc = core_tables(c, xd, dinv, plan)
        m = {"xdq": xd, "xdT": xdT, "winvc": winvc, "dinvc": dvc,
             "wgT": wgT, "w1T": w1T, "w2tp": w2tp, "w3c": w3c,
             "idx0": in_extra[c]["idx0"], "idx1": in_extra[c]["idx1"],
             "dsb0": in_extra[c]["dsb0"], "dsb1": in_extra[c]["dsb1"]}
        if bias_info[0]:
            m["bgr"] = bg.reshape(1, 128)
        if bias_info[1]:
            m["b1c"] = b1.reshape(4, 128).T.astype(np.float32).copy()
        if bias_info[2]:
            m["b2c"] = b2.reshape(2, 128).T.astype(np.float32).copy()
        in_maps.append(m)

    res = run_bass_kernel_spmd(nc, in_maps, list(range(NCORES)), trace=TRACE)
    global LAST_EXEC_NS, LAST_RESULT
    LAST_EXEC_NS = res.exec_time_ns
    LAST_RESULT = res
    return res.results[0]["out"].reshape(1).astype(np.float32)


# revision 42
# speedup vs baseline: 1.0554x; 1.0046x over previous
"""GCN critic network kernel for 8 TRN2 NeuronCores.

Strategy (dst-shard, no message-table collective):
  The GCNConv linear commutes with the segment sum:
      out[d] = dinv_d * (sum_{s in N(d)} dinv_s * x[s]) @ Wg.T + bg
  so each core gathers pre-scaled raw rows xd = dinv*x (host-prepped bf16
  table in DRAM, a kernel input) for its own dst shard's edges and applies
  Wg once per 128-dst block after aggregation. This removes the y-table
  AllGather of the previous design entirely; the only collective left is
  the tiny [1,128] AllReduce of the pooled vector.

  - dst nodes sharded 6250/core (padded 6272 = 49 blocks of 128).
  - Edge messages: dma_gather of 256B bf16 rows (int16 indices, node table
    split in two <=32768-row groups), segment-summed per dst block via
    one-hot matmul accumulation in PSUM:  psST[fin, slot] += seg.T @ oh.
  - One-hot built per chunk with a single DVE tensor_scalar is_equal
    (iota row constant vs per-partition dst-slot scalar) - hits the
    packed-2-byte DVE fast path.
  - Self-loops folded in as one extra matmul per block from the resident
    xdT slice (no gathered self rows).
  - h = relu(dinv * (aggT.T @ WgT)); v = ones@h blocks + colsum(x_own);
    AllReduce v; tiny MLP head with host-pretransposed weights.
"""

import os
import numpy as np
import ml_dtypes

BF16 = ml_dtypes.bfloat16
N = 50000
E = 800000
D = 128
NCORES = 8
NPC = 6250          # dst nodes per core
NPAD = 6272         # padded (49 * 128)
NB = NPAD // 128    # dst blocks per core
GRP = 32768         # int16 index-group boundary (table-row space)
SEGC = int(os.environ.get("KB_SEGC", "8"))   # chunks per gather call
DDS = int(os.environ.get("KB_DDS", "65536"))
PADSLOT = 300.0     # dst-slot sentinel for padding rows (one-hot miss)

DEBUG_BLOCKS = (int(os.environ["KB_DEBUG_BLOCKS"])
                if "KB_DEBUG_BLOCKS" in os.environ else None)
SKIP_MLP = bool(os.environ.get("KB_SKIP_MLP"))
SKIP_MAIN = bool(os.environ.get("KB_SKIP_MAIN"))


def _lpt_assign(src, dst):
    """Per-core 2D-LPT node->(block, slot) map balancing per-(group, block)
    edge counts across cores; shrinks the uniform chunk-count padding."""
    c0 = np.bincount(dst[src < GRP], minlength=N)
    c1 = np.bincount(dst[src >= GRP], minlength=N)
    node_blk = np.empty(N, np.int64)
    node_slot = np.empty(N, np.int64)
    # the last-processed block gets a reduced target so the tail drain
    # after the final gather is short (lowest-degree nodes settle there)
    FTAIL = 0.4
    scale = np.ones(NB)
    scale[NB - 1] = FTAIL
    for c in range(NCORES):
        nodes = np.arange(c * NPC, (c + 1) * NPC)
        a0, a1 = c0[nodes].astype(np.float64), c1[nodes].astype(np.float64)
        order = np.argsort(-(a0 + a1), kind="stable")
        T0 = np.maximum(a0.sum() * scale / scale.sum(), 1.0)
        T1 = np.maximum(a1.sum() * scale / scale.sum(), 1.0)
        w0 = np.zeros(NB)
        w1 = np.zeros(NB)
        ns = np.zeros(NB, np.int64)
        for j in order:
            pen = np.where(ns < 128,
                           np.maximum((w0 + a0[j]) / T0, (w1 + a1[j]) / T1),
                           np.inf)
            b = int(np.argmin(pen))
            node_blk[nodes[j]] = b
            node_slot[nodes[j]] = ns[b]
            w0[b] += a0[j]
            w1[b] += a1[j]
            ns[b] += 1
    return node_blk, node_slot


def _prep(edge_index, use_lpt=True):
    """Host-side graph prep: per-core chunked edge layout + uniform plan."""
    src = np.asarray(edge_index[0]).astype(np.int64)
    dst = np.asarray(edge_index[1]).astype(np.int64)

    deg = np.bincount(dst, minlength=N).astype(np.float64) + 1.0
    dinv = (1.0 / np.sqrt(deg)).astype(np.float32)

    if use_lpt and not os.environ.get("KB_NOLPT"):
        node_blk, node_slot = _lpt_assign(src, dst)
    else:
        local = np.arange(N) % NPC
        node_blk = local >> 7
        node_slot = local & 127

    per_core = []
    cnt = np.zeros((NCORES, 2, NB), dtype=np.int64)
    for c in range(NCORES):
        lo, hi = c * NPC, (c + 1) * NPC
        m = (dst >= lo) & (dst < hi)
        es = src[m]
        ds = dst[m]
        g = (es >= GRP).astype(np.int64)
        blk = node_blk[ds]
        slot = node_slot[ds]
        # sort by (group, block, src) - src order improves HBM locality
        order = np.lexsort((es, blk, g))
        es, slot, g, blk = es[order], slot[order], g[order], blk[order]
        np.add.at(cnt[c], (g, blk), 1)
        per_core.append((es, slot, g, blk))

    # uniform chunk counts per (group, block) across cores (SPMD program)
    ct = np.ceil(cnt.max(axis=0) / 128.0).astype(np.int64)  # [2, NB]
    CH = ct.sum(axis=1)                                     # chunks per group
    off = np.zeros((2, NB), dtype=np.int64)
    off[:, 1:] = np.cumsum(ct, axis=1)[:, :-1]

    def wrap_idx(loc):
        nch = len(loc) // 128
        cols = []
        for s0 in range(0, nch, SEGC):
            seg = loc[s0 * 128: min(nch, s0 + SEGC) * 128]
            a = seg.reshape(-1, 16).T.astype(np.int16)        # [16, cols_s]
            cols.append(np.tile(a, (8, 1)))                   # [128, cols_s]
        return np.concatenate(cols, axis=1)

    in_extra = []
    for c in range(NCORES):
        es, slot, gs, blks = per_core[c]
        d = {}
        for g in range(2):
            loc = np.zeros(CH[g] * 128, dtype=np.int64)
            slo = np.full(CH[g] * 128, PADSLOT, dtype=np.float64)
            sel = gs == g
            ep, sl, bl = es[sel] - g * GRP, slot[sel], blks[sel]
            bstart = np.zeros(NB, dtype=np.int64)
            bstart[1:] = np.cumsum(np.bincount(bl, minlength=NB))[:-1]
            pos = off[g][bl] * 128 + (np.arange(len(ep)) - bstart[bl])
            loc[pos] = ep
            slo[pos] = sl
            d[f"idx{g}"] = wrap_idx(loc)
            d[f"dsb{g}"] = slo.reshape(-1, 128).T.astype(BF16)
        in_extra.append(d)

    plan = {"ct": ct, "CH": CH, "off": off,
            "nblk": node_blk, "nslot": node_slot}
    return dinv, in_extra, plan


def core_tables(c, xd, dinv, plan):
    """Per-core dst-side tables in the (possibly LPT-permuted) layout."""
    nblk, nslot = plan["nblk"], plan["nslot"]
    lo, hi = c * NPC, (c + 1) * NPC
    pos = nblk[lo:hi] * 128 + nslot[lo:hi]
    xdT = np.zeros((128, NPAD), dtype=BF16)
    xdT[:, pos] = xd[lo:hi].T
    dv = np.zeros(NPAD, dtype=np.float32)
    dv[pos] = dinv[lo:hi]
    dvc = dv.reshape(NB, 128).T.copy()
    wv = np.zeros(NPAD, dtype=np.float32)
    wv[pos] = 1.0 / dinv[lo:hi]
    winvc = wv.reshape(NB, 128).T.copy()
    return xdT, dvc, winvc


def _build(plan, bias_info):
    import concourse.bacc as bacc
    import concourse.tile as tile
    from concourse import mybir

    f32 = mybir.dt.float32
    f16 = mybir.dt.bfloat16
    i16 = mybir.dt.int16
    Alu = mybir.AluOpType
    Act = mybir.ActivationFunctionType

    ct, CH, off = plan["ct"], plan["CH"], plan["off"]
    has_bg, has_b1, has_b2, b3val = bias_info

    nc = bacc.Bacc("TRN2", target_bir_lowering=False, debug=False,
                   num_devices=NCORES,
                   dynamic_dma_scratch_size=DDS)

    def din(name, shape, dt=f32):
        return nc.dram_tensor(name, list(shape), dt, kind="ExternalInput")

    xdq_d = din("xdq", [N, D], f16)          # dinv*x, full table (bf16)
    xdT_d = din("xdT", [128, NPAD], f16)     # (dinv*x)[own].T
    dinvc_d = din("dinvc", [128, NB])
    winv_d = din("winvc", [128, NB])         # 1/dinv per (slot, block)
    wgT_d = din("wgT", [D, D], f16)          # Wg.T
    w1T_d = din("w1T", [128, 512])           # W1.T
    w2tp_d = din("w2tp", [128, 1024])        # W2.T row-blocks packed
    w3c_d = din("w3c", [128, 2])             # W3.T column chunks
    idx_d = [din(f"idx{g}", [128, int(CH[g]) * 8], i16) for g in range(2)]
    dsb_d = [din(f"dsb{g}", [128, int(CH[g])], f16) for g in range(2)]
    bgr_d = din("bgr", [1, 128]) if has_bg else None
    b1c_d = din("b1c", [128, 4]) if has_b1 else None
    b2c_d = din("b2c", [128, 2]) if has_b2 else None
    out_d = nc.dram_tensor("out", [1, 1], f32, kind="ExternalOutput")

    vb = nc.dram_tensor("vb", [1, 128], f32)
    vr = nc.dram_tensor("vr", [NCORES, 128], f32, addr_space="Shared")

    RG = [list(range(NCORES))]

    with tile.TileContext(nc) as tc:
        with (
            tc.tile_pool(name="const", bufs=1) as cpool,
            tc.tile_pool(name="seg", bufs=10) as segpool,
            tc.tile_pool(name="oh", bufs=10) as ohpool,
            tc.tile_pool(name="agg", bufs=3) as apool,
            tc.tile_pool(name="hb", bufs=3) as hpool,
            tc.tile_pool(name="mlp", bufs=1) as mpool,
            tc.tile_pool(name="psv", bufs=1, space="PSUM") as pvpool,
            tc.tile_pool(name="psS", bufs=3, space="PSUM") as pspool,
            tc.tile_pool(name="psH", bufs=2, space="PSUM") as phpool,
            tc.tile_pool(name="psT", bufs=1, space="PSUM") as ptpool,
        ):
            # ---- prefetch: gather-critical tables first ----
            # idx0 split so the first gather segment unblocks early
            idx_t = []
            it0 = cpool.tile([128, int(CH[0]) * 8], i16, tag="idx0")
            c_split = min(4 * SEGC * 8, int(CH[0]) * 8)
            nc.sync.dma_start(it0[:, 0:c_split], idx_d[0][:, 0:c_split])
            dsbh_t = []
            dh0 = cpool.tile([128, int(CH[0])], f16, tag="dsbh0")
            nc.sync.dma_start(dh0[:], dsb_d[0][:])
            nc.sync.dma_start(it0[:, c_split:int(CH[0]) * 8],
                              idx_d[0][:, c_split:int(CH[0]) * 8])
            idx_t.append(it0)
            dsbh_t.append(dh0)
            it1 = cpool.tile([128, int(CH[1]) * 8], i16, tag="idx1")
            nc.sync.dma_start(it1[:], idx_d[1][:])
            idx_t.append(it1)
            dh1 = cpool.tile([128, int(CH[1])], f16, tag="dsbh1")
            nc.sync.dma_start(dh1[:], dsb_d[1][:])
            dsbh_t.append(dh1)
            dinv_t = cpool.tile([128, NB], f32)
            nc.sync.dma_start(dinv_t[:], dinvc_d[:])
            winv_t = cpool.tile([128, NB], f32, tag="winv")
            nc.sync.dma_start(winv_t[:], winv_d[:])
            wgT_t = cpool.tile([128, 128], f16, tag="wgT")
            nc.sync.dma_start(wgT_t[:], wgT_d[:])
            xdT_t = cpool.tile([128, NPAD], f16, tag="xdT")
            nc.sync.dma_start(xdT_t[:], xdT_d[:])

            # on-chip constants (no DMA): iota row, identities, ones
            iota_t = cpool.tile([128, 128], f16)
            nc.gpsimd.iota(iota_t[:], [[1, 128]], channel_multiplier=0,
                           allow_small_or_imprecise_dtypes=True)
            pcol_t = cpool.tile([128, 1], f32, tag="pcol")
            nc.gpsimd.iota(pcol_t[:], [[0, 1]], channel_multiplier=1,
                           allow_small_or_imprecise_dtypes=True)
            ones_t = cpool.tile([128, 1], f16)
            nc.vector.memset(ones_t[:], 1.0)
            # dsb tables cast to f32 (is_equal scalar operand must be f32)
            dsb_t = []
            for g in range(2):
                dt_ = cpool.tile([128, int(CH[g])], f32, tag=f"dsb{g}")
                nc.vector.tensor_copy(dt_[:], dsbh_t[g][:])
                dsb_t.append(dt_)
            idh_t = cpool.tile([128, 128], f16)
            nc.vector.tensor_scalar(idh_t[:], iota_t[:], pcol_t[:],
                                    None, Alu.is_equal)
            idf_t = cpool.tile([128, 128], f32)
            nc.vector.tensor_scalar(idf_t[:], iota_t[:], pcol_t[:],
                                    None, Alu.is_equal)

            if has_bg:
                bgr_t = cpool.tile([1, 128], f32, tag="bgr")
                nc.sync.dma_start(bgr_t[:], bgr_d[:])
                # materialize bg as [128,128] via ones-outer-product
                ones_f = cpool.tile([128, 1], f32, tag="onesf")
                nc.vector.memset(ones_f[:], 1.0)
                psb = ptpool.tile([128, 128], f32, tag="pst")
                nc.tensor.matmul(psb[:], ones_f[:], bgr_t[:],
                                 start=True, stop=True)
                bg_t = cpool.tile([128, 128], f32, tag="bgt")
                nc.vector.tensor_copy(bg_t[:], psb[:])

            # ---- main: gather + one-hot segment-sum + W apply ----
            ytab = [xdq_d[0:GRP, :], xdq_d[GRP:N, :]]
            nseg = [int(np.ceil(CH[g] / SEGC)) for g in range(2)]
            seg_tiles = [[None] * nseg[g] for g in range(2)]
            oh_tiles = [[None] * nseg[g] for g in range(2)]
            seg_ptr = [0, 0]

            def ensure_seg(g, s):
                while seg_ptr[g] <= s:
                    si = seg_ptr[g]
                    ncols = min(SEGC, int(CH[g]) - si * SEGC)
                    tl = segpool.tile([128, ncols, 128], f16, tag="seg")
                    nidx = ncols * 128
                    nc.gpsimd.dma_gather(
                        tl[:], ytab[g], idx_t[g][:, si * (SEGC * 8):
                                                 si * (SEGC * 8) + ncols * 8],
                        num_idxs=nidx, num_idxs_reg=nidx,
                        elem_size=128, elem_step=128)
                    seg_tiles[g][si] = tl
                    oh = ohpool.tile([128, ncols, 128], f16, tag="oh")
                    for cc in range(ncols):
                        nc.vector.tensor_scalar(
                            oh[:, cc, :], iota_t[:],
                            dsb_t[g][:, si * SEGC + cc: si * SEGC + cc + 1],
                            None, Alu.is_equal)
                    oh_tiles[g][si] = oh
                    seg_ptr[g] += 1

            psv = pvpool.tile([1, 128], f32, tag="psv")
            first_v = [True]
            nblk = (0 if SKIP_MAIN else
                    (NB if DEBUG_BLOCKS is None else DEBUG_BLOCKS))
            for b in range(nblk):
                # residual colsum via xdT: x rows = (1/dinv) * xd rows.
                # Issued at iteration top so block b's chain overlaps its
                # own gather/matmul stream; psv group closes on final vacc.
                pstx = ptpool.tile([128, 128], f16, tag="pst")
                nc.tensor.transpose(pstx[:], xdT_t[:, b * 128:(b + 1) * 128],
                                    idh_t[:])
                xdr = hpool.tile([128, 128], f32, tag="xdr")
                nc.scalar.activation(xdr[:], pstx[:], Act.Copy)
                nc.tensor.matmul(psv[:], winv_t[:, b:b + 1], xdr[:],
                                 start=first_v[0], stop=False,
                                 skip_group_check=True)
                first_v[0] = False
                psS = pspool.tile([128, 128], f32, tag="psS")
                tot = int(ct[0][b] + ct[1][b])
                k = 0
                for g in range(2):
                    for j in range(int(ct[g][b])):
                        ci = int(off[g][b]) + j
                        s, col = divmod(ci, SEGC)
                        ensure_seg(g, s)
                        nc.tensor.matmul(psS[:], seg_tiles[g][s][:, col, :],
                                         oh_tiles[g][s][:, col, :],
                                         start=(k == 0), stop=(k == tot - 1))
                        k += 1
                # aggT: [fin, slot] PSUM -> SBUF bf16 (ACT engine)
                aggT = apool.tile([128, 128], f16, tag="agg")
                if tot == 0:
                    nc.vector.memset(aggT[:], 0.0)
                else:
                    nc.scalar.activation(aggT[:], psS[:], Act.Copy)
                # W apply + self-loop term
                psH = phpool.tile([128, 128], f32, tag="psH")
                nc.tensor.matmul(psH[:], aggT[:], wgT_t[:],
                                 start=True, stop=False)
                nc.tensor.matmul(psH[:], xdT_t[:, b * 128:(b + 1) * 128],
                                 wgT_t[:], start=False, stop=True)
                hb = hpool.tile([128, 128], f16)
                if has_bg:
                    tmp = hpool.tile([128, 128], f32, tag="tmp")
                    nc.vector.tensor_scalar(tmp[:], psH[:],
                                            dinv_t[:, b:b + 1], None, Alu.mult)
                    nc.vector.tensor_tensor(tmp[:], tmp[:], bg_t[:], Alu.add)
                    nc.scalar.activation(hb[:], tmp[:], Act.Relu)
                    if b == NB - 1:
                        nc.vector.memset(hb[NPC - (NB - 1) * 128:128, :], 0.0)
                else:
                    nc.scalar.activation(hb[:], psH[:], Act.Relu,
                                         scale=dinv_t[:, b:b + 1])
                nc.tensor.matmul(psv[:], ones_t[:], hb[:],
                                 start=False, stop=(b == nblk - 1),
                                 skip_group_check=True)

            # late-use loads (overlap the tail of the gather stream)
            w1T_t = mpool.tile([128, 512], f32, tag="w1T")
            nc.sync.dma_start(w1T_t[:], w1T_d[:])
            w2tp_t = mpool.tile([128, 1024], f32, tag="w2tp")
            nc.sync.dma_start(w2tp_t[:], w2tp_d[:])
            w3c_t = mpool.tile([128, 2], f32, tag="w3c")
            nc.sync.dma_start(w3c_t[:], w3c_d[:])
            if has_b1:
                b1c_t = mpool.tile([128, 4], f32, tag="b1c")
                nc.sync.dma_start(b1c_t[:], b1c_d[:])
            if has_b2:
                b2c_t = mpool.tile([128, 2], f32, tag="b2c")
                nc.sync.dma_start(b2c_t[:], b2c_d[:])

            # ---- v exchange: AllGather partial rows, sum on-chip ----
            vrow = mpool.tile([1, 128], f32, tag="vrow")
            if nblk == 0:
                nc.vector.memset(vrow[:], 0.0)
            else:
                nc.scalar.copy(vrow[:], psv[:])
            nc.sync.dma_start(vb[:], vrow[:])
            nc.gpsimd.collective_compute(
                "AllGather", Alu.bypass, replica_groups=RG,
                ins=[vb[:]], outs=[vr[:]])
            vgt = mpool.tile([NCORES, 128], f32, tag="vgt")
            nc.sync.dma_start(vgt[:], vr[:])
            ones8 = mpool.tile([NCORES, 1], f32, tag="ones8")
            nc.vector.memset(ones8[:], 1.0)
            # ---- MLP head ----
            if SKIP_MLP:
                psv2 = pvpool.tile([1, 128], f32, tag="psv")
                nc.tensor.matmul(psv2[:], ones8[:], vgt[:],
                                 start=True, stop=True)
                vfull = mpool.tile([1, 128], f32, tag="vfull")
                nc.scalar.copy(vfull[:], psv2[:])
                nc.sync.dma_start(out_d[:], vfull[0:1, 0:1])
            else:
                psv3 = ptpool.tile([128, 1], f32, tag="pst")
                nc.tensor.matmul(psv3[:], vgt[:], ones8[:],
                                 start=True, stop=True)
                vcol = mpool.tile([128, 1], f32, tag="vcol")
                nc.vector.tensor_copy(vcol[:], psv3[:])

                a1c = []
                for m in range(4):
                    ps1 = ptpool.tile([128, 1], f32, tag="pst")
                    nc.tensor.matmul(ps1[:], w1T_t[:, m * 128:(m + 1) * 128],
                                     vcol[:], start=True, stop=True)
                    a1 = mpool.tile([128, 1], f32, tag=f"a1c{m}")
                    if has_b1:
                        nc.scalar.activation(a1[:], ps1[:], Act.Relu,
                                             bias=b1c_t[:, m:m + 1])
                    else:
                        nc.scalar.activation(a1[:], ps1[:], Act.Relu)
                    a1c.append(a1)

                a2c = []
                for m in range(2):
                    ps2 = ptpool.tile([128, 1], f32, tag="pst")
                    for kk in range(4):
                        nc.tensor.matmul(
                            ps2[:],
                            w2tp_t[:, kk * 256 + m * 128:
                                   kk * 256 + (m + 1) * 128],
                            a1c[kk][:], start=(kk == 0), stop=(kk == 3))
                    a2 = mpool.tile([128, 1], f32, tag=f"a2c{m}")
                    if has_b2:
                        nc.scalar.activation(a2[:], ps2[:], Act.Relu,
                                             bias=b2c_t[:, m:m + 1])
                    else:
                        nc.scalar.activation(a2[:], ps2[:], Act.Relu)
                    a2c.append(a2)

                ps3 = ptpool.tile([1, 1], f32, tag="ps3")
                for kk in range(2):
                    nc.tensor.matmul(ps3[:], w3c_t[:, kk:kk + 1], a2c[kk][:],
                                     start=(kk == 0), stop=(kk == 1))
                ot = mpool.tile([1, 1], f32, tag="ot")
                nc.scalar.activation(ot[:], ps3[:], Act.Copy,
                                     bias=float(b3val))
                nc.sync.dma_start(out_d[:], ot[:])

    nc.compile()
    return nc


TRACE = False
LAST_EXEC_NS = None
LAST_RESULT = None


def kernel(**inputs):
    from concourse.bass_utils import run_bass_kernel_spmd

    x = np.asarray(inputs["x"], dtype=np.float32)
    Wg = np.asarray(inputs["Wg"], dtype=np.float32)
    bg = np.asarray(inputs["bg"], dtype=np.float32)
    W1 = np.asarray(inputs["W1"], dtype=np.float32)
    b1 = np.asarray(inputs["b1"], dtype=np.float32)
    W2 = np.asarray(inputs["W2"], dtype=np.float32)
    b2 = np.asarray(inputs["b2"], dtype=np.float32)
    W3 = np.asarray(inputs["W3"], dtype=np.float32)
    b3 = np.asarray(inputs["b3"], dtype=np.float32)

    bias_info = (bool(bg.any()), bool(b1.any()), bool(b2.any()),
                 float(b3.reshape(-1)[0]))
    # LPT layout leaves empty slots scattered across blocks, which the
    # bg!=0 path cannot mask; fall back to the sequential layout there.
    dinv, in_extra, plan = _prep(inputs["edge_index"],
                                 use_lpt=not bias_info[0])
    nc = _build(plan, bias_info)

    xd = (dinv[:, None] * x).astype(BF16)           # [N, D] table
    wgT = Wg.T.astype(BF16).copy()
    w1T = W1.T.astype(np.float32).copy()            # [128, 512]
    w2tp = np.concatenate([W2.T[k * 128:(k + 1) * 128] for k in range(4)],
                          axis=1).astype(np.float32).copy()  # [128, 1024]
    w3c = W3.reshape(2, 128).T.astype(np.float32).copy()     # [128, 2]

    in_maps = []
    for c in range(NCORES):
        xdT, dvc, winvc = core_tables(c, xd, dinv, plan)
        m = {"xdq": xd, "xdT": xdT, "winvc": winvc, "dinvc": dvc,
             "wgT": wgT, "w1T": w1T, "w2tp": w2tp, "w3c": w3c,
             "idx0": in_extra[c]["idx0"], "idx1": in_extra[c]["idx1"],
             "dsb0": in_extra[c]["dsb0"], "dsb1": in_extra[c]["dsb1"]}
        if bias_info[0]:
            m["bgr"] = bg.reshape(1, 128)
        if bias_info[1]:
            m["b1c"] = b1.reshape(4, 128).T.astype(np.float32).copy()
        if bias_info[2]:
            m["b2c"] = b2.reshape(2, 128).T.astype(np.float32).copy()
        in_maps.append(m)

    res = run_bass_kernel_spmd(nc, in_maps, list(range(NCORES)), trace=TRACE)
    global LAST_EXEC_NS, LAST_RESULT
    LAST_EXEC_NS = res.exec_time_ns
    LAST_RESULT = res
    return res.results[0]["out"].reshape(1).astype(np.float32)


# revision 44
# speedup vs baseline: 1.0601x; 1.0045x over previous
"""GCN critic network kernel for 8 TRN2 NeuronCores.

Strategy (dst-shard, no message-table collective):
  The GCNConv linear commutes with the segment sum:
      out[d] = dinv_d * (sum_{s in N(d)} dinv_s * x[s]) @ Wg.T + bg
  so each core gathers pre-scaled raw rows xd = dinv*x (host-prepped bf16
  table in DRAM, a kernel input) for its own dst shard's edges and applies
  Wg once per 128-dst block after aggregation. This removes the y-table
  AllGather of the previous design entirely; the only collective left is
  the tiny [1,128] AllReduce of the pooled vector.

  - dst nodes sharded 6250/core (padded 6272 = 49 blocks of 128).
  - Edge messages: dma_gather of 256B bf16 rows (int16 indices, node table
    split in two <=32768-row groups), segment-summed per dst block via
    one-hot matmul accumulation in PSUM:  psST[fin, slot] += seg.T @ oh.
  - One-hot built per chunk with a single DVE tensor_scalar is_equal
    (iota row constant vs per-partition dst-slot scalar) - hits the
    packed-2-byte DVE fast path.
  - Self-loops folded in as one extra matmul per block from the resident
    xdT slice (no gathered self rows).
  - h = relu(dinv * (aggT.T @ WgT)); v = ones@h blocks + colsum(x_own);
    AllReduce v; tiny MLP head with host-pretransposed weights.
"""

import os
import numpy as np
import ml_dtypes

BF16 = ml_dtypes.bfloat16
N = 50000
E = 800000
D = 128
NCORES = 8
NPC = 6250          # dst nodes per core
NPAD = 6272         # padded (49 * 128)
NB = NPAD // 128    # dst blocks per core
GRP = 32768         # int16 index-group boundary (table-row space)
SEGC = int(os.environ.get("KB_SEGC", "8"))   # chunks per gather call
DDS = int(os.environ.get("KB_DDS", "65536"))
PADSLOT = 300.0     # dst-slot sentinel for padding rows (one-hot miss)

DEBUG_BLOCKS = (int(os.environ["KB_DEBUG_BLOCKS"])
                if "KB_DEBUG_BLOCKS" in os.environ else None)
SKIP_MLP = bool(os.environ.get("KB_SKIP_MLP"))
SKIP_MAIN = bool(os.environ.get("KB_SKIP_MAIN"))


def _lpt_assign(src, dst):
    """Per-core 2D-LPT node->(block, slot) map balancing per-(group, block)
    edge counts across cores; shrinks the uniform chunk-count padding."""
    c0 = np.bincount(dst[src < GRP], minlength=N)
    c1 = np.bincount(dst[src >= GRP], minlength=N)
    node_blk = np.empty(N, np.int64)
    node_slot = np.empty(N, np.int64)
    # the last-processed block gets a reduced target so the tail drain
    # after the final gather is short (lowest-degree nodes settle there)
    FTAIL = 0.25
    scale = np.ones(NB)
    scale[NB - 1] = FTAIL
    for c in range(NCORES):
        nodes = np.arange(c * NPC, (c + 1) * NPC)
        a0, a1 = c0[nodes].astype(np.float64), c1[nodes].astype(np.float64)
        order = np.argsort(-(a0 + a1), kind="stable")
        T0 = np.maximum(a0.sum() * scale / scale.sum(), 1.0)
        T1 = np.maximum(a1.sum() * scale / scale.sum(), 1.0)
        w0 = np.zeros(NB)
        w1 = np.zeros(NB)
        ns = np.zeros(NB, np.int64)
        for j in order:
            pen = np.where(ns < 128,
                           np.maximum((w0 + a0[j]) / T0, (w1 + a1[j]) / T1),
                           np.inf)
            b = int(np.argmin(pen))
            node_blk[nodes[j]] = b
            node_slot[nodes[j]] = ns[b]
            w0[b] += a0[j]
            w1[b] += a1[j]
            ns[b] += 1
    return node_blk, node_slot


def _prep(edge_index, use_lpt=True):
    """Host-side graph prep: per-core chunked edge layout + uniform plan."""
    src = np.asarray(edge_index[0]).astype(np.int64)
    dst = np.asarray(edge_index[1]).astype(np.int64)

    deg = np.bincount(dst, minlength=N).astype(np.float64) + 1.0
    dinv = (1.0 / np.sqrt(deg)).astype(np.float32)

    if use_lpt and not os.environ.get("KB_NOLPT"):
        node_blk, node_slot = _lpt_assign(src, dst)
    else:
        local = np.arange(N) % NPC
        node_blk = local >> 7
        node_slot = local & 127

    per_core = []
    cnt = np.zeros((NCORES, 2, NB), dtype=np.int64)
    for c in range(NCORES):
        lo, hi = c * NPC, (c + 1) * NPC
        m = (dst >= lo) & (dst < hi)
        es = src[m]
        ds = dst[m]
        g = (es >= GRP).astype(np.int64)
        blk = node_blk[ds]
        slot = node_slot[ds]
        # sort by (group, block, src) - src order improves HBM locality
        order = np.lexsort((es, blk, g))
        es, slot, g, blk = es[order], slot[order], g[order], blk[order]
        np.add.at(cnt[c], (g, blk), 1)
        per_core.append((es, slot, g, blk))

    # uniform chunk counts per (group, block) across cores (SPMD program)
    ct = np.ceil(cnt.max(axis=0) / 128.0).astype(np.int64)  # [2, NB]
    CH = ct.sum(axis=1)                                     # chunks per group
    off = np.zeros((2, NB), dtype=np.int64)
    off[:, 1:] = np.cumsum(ct, axis=1)[:, :-1]

    def wrap_idx(loc):
        nch = len(loc) // 128
        cols = []
        for s0 in range(0, nch, SEGC):
            seg = loc[s0 * 128: min(nch, s0 + SEGC) * 128]
            a = seg.reshape(-1, 16).T.astype(np.int16)        # [16, cols_s]
            cols.append(np.tile(a, (8, 1)))                   # [128, cols_s]
        return np.concatenate(cols, axis=1)

    in_extra = []
    for c in range(NCORES):
        es, slot, gs, blks = per_core[c]
        d = {}
        for g in range(2):
            loc = np.zeros(CH[g] * 128, dtype=np.int64)
            slo = np.full(CH[g] * 128, PADSLOT, dtype=np.float64)
            sel = gs == g
            ep, sl, bl = es[sel] - g * GRP, slot[sel], blks[sel]
            bstart = np.zeros(NB, dtype=np.int64)
            bstart[1:] = np.cumsum(np.bincount(bl, minlength=NB))[:-1]
            pos = off[g][bl] * 128 + (np.arange(len(ep)) - bstart[bl])
            loc[pos] = ep
            slo[pos] = sl
            d[f"idx{g}"] = wrap_idx(loc)
            d[f"dsb{g}"] = slo.reshape(-1, 128).T.astype(BF16)
        in_extra.append(d)

    plan = {"ct": ct, "CH": CH, "off": off,
            "nblk": node_blk, "nslot": node_slot}
    return dinv, in_extra, plan


def core_tables(c, xd, dinv, plan):
    """Per-core dst-side tables in the (possibly LPT-permuted) layout."""
    nblk, nslot = plan["nblk"], plan["nslot"]
    lo, hi = c * NPC, (c + 1) * NPC
    pos = nblk[lo:hi] * 128 + nslot[lo:hi]
    xdT = np.zeros((128, NPAD), dtype=BF16)
    xdT[:, pos] = xd[lo:hi].T
    dv = np.zeros(NPAD, dtype=np.float32)
    dv[pos] = dinv[lo:hi]
    dvc = dv.reshape(NB, 128).T.copy()
    wv = np.zeros(NPAD, dtype=np.float32)
    wv[pos] = 1.0 / dinv[lo:hi]
    winvc = wv.reshape(NB, 128).T.copy()
    return xdT, dvc, winvc


def _build(plan, bias_info):
    import concourse.bacc as bacc
    import concourse.tile as tile
    from concourse import mybir

    f32 = mybir.dt.float32
    f16 = mybir.dt.bfloat16
    i16 = mybir.dt.int16
    Alu = mybir.AluOpType
    Act = mybir.ActivationFunctionType

    ct, CH, off = plan["ct"], plan["CH"], plan["off"]
    has_bg, has_b1, has_b2, b3val = bias_info

    nc = bacc.Bacc("TRN2", target_bir_lowering=False, debug=False,
                   num_devices=NCORES,
                   dynamic_dma_scratch_size=DDS)

    def din(name, shape, dt=f32):
        return nc.dram_tensor(name, list(shape), dt, kind="ExternalInput")

    xdq_d = din("xdq", [N, D], f16)          # dinv*x, full table (bf16)
    xdT_d = din("xdT", [128, NPAD], f16)     # (dinv*x)[own].T
    dinvc_d = din("dinvc", [128, NB])
    winv_d = din("winvc", [128, NB])         # 1/dinv per (slot, block)
    wgT_d = din("wgT", [D, D], f16)          # Wg.T
    w1T_d = din("w1T", [128, 512])           # W1.T
    w2tp_d = din("w2tp", [128, 1024])        # W2.T row-blocks packed
    w3c_d = din("w3c", [128, 2])             # W3.T column chunks
    idx_d = [din(f"idx{g}", [128, int(CH[g]) * 8], i16) for g in range(2)]
    dsb_d = [din(f"dsb{g}", [128, int(CH[g])], f16) for g in range(2)]
    bgr_d = din("bgr", [1, 128]) if has_bg else None
    b1c_d = din("b1c", [128, 4]) if has_b1 else None
    b2c_d = din("b2c", [128, 2]) if has_b2 else None
    out_d = nc.dram_tensor("out", [1, 1], f32, kind="ExternalOutput")

    vb = nc.dram_tensor("vb", [1, 128], f32)
    vr = nc.dram_tensor("vr", [NCORES, 128], f32, addr_space="Shared")

    RG = [list(range(NCORES))]

    with tile.TileContext(nc) as tc:
        with (
            tc.tile_pool(name="const", bufs=1) as cpool,
            tc.tile_pool(name="seg", bufs=10) as segpool,
            tc.tile_pool(name="oh", bufs=10) as ohpool,
            tc.tile_pool(name="agg", bufs=3) as apool,
            tc.tile_pool(name="hb", bufs=3) as hpool,
            tc.tile_pool(name="mlp", bufs=1) as mpool,
            tc.tile_pool(name="psv", bufs=1, space="PSUM") as pvpool,
            tc.tile_pool(name="psS", bufs=3, space="PSUM") as pspool,
            tc.tile_pool(name="psH", bufs=2, space="PSUM") as phpool,
            tc.tile_pool(name="psT", bufs=1, space="PSUM") as ptpool,
        ):
            # ---- prefetch: gather-critical tables first ----
            # idx0 split so the first gather segment unblocks early
            idx_t = []
            it0 = cpool.tile([128, int(CH[0]) * 8], i16, tag="idx0")
            c_split = min(4 * SEGC * 8, int(CH[0]) * 8)
            nc.sync.dma_start(it0[:, 0:c_split], idx_d[0][:, 0:c_split])
            dsbh_t = []
            dh0 = cpool.tile([128, int(CH[0])], f16, tag="dsbh0")
            nc.sync.dma_start(dh0[:], dsb_d[0][:])
            nc.sync.dma_start(it0[:, c_split:int(CH[0]) * 8],
                              idx_d[0][:, c_split:int(CH[0]) * 8])
            idx_t.append(it0)
            dsbh_t.append(dh0)
            it1 = cpool.tile([128, int(CH[1]) * 8], i16, tag="idx1")
            nc.sync.dma_start(it1[:], idx_d[1][:])
            idx_t.append(it1)
            dh1 = cpool.tile([128, int(CH[1])], f16, tag="dsbh1")
            nc.sync.dma_start(dh1[:], dsb_d[1][:])
            dsbh_t.append(dh1)
            dinv_t = cpool.tile([128, NB], f32)
            nc.sync.dma_start(dinv_t[:], dinvc_d[:])
            winv_t = cpool.tile([128, NB], f32, tag="winv")
            nc.sync.dma_start(winv_t[:], winv_d[:])
            wgT_t = cpool.tile([128, 128], f16, tag="wgT")
            nc.sync.dma_start(wgT_t[:], wgT_d[:])
            xdT_t = cpool.tile([128, NPAD], f16, tag="xdT")
            nc.sync.dma_start(xdT_t[:], xdT_d[:])

            # on-chip constants (no DMA): iota row, identities, ones
            iota_t = cpool.tile([128, 128], f16)
            nc.gpsimd.iota(iota_t[:], [[1, 128]], channel_multiplier=0,
                           allow_small_or_imprecise_dtypes=True)
            pcol_t = cpool.tile([128, 1], f32, tag="pcol")
            nc.gpsimd.iota(pcol_t[:], [[0, 1]], channel_multiplier=1,
                           allow_small_or_imprecise_dtypes=True)
            ones_t = cpool.tile([128, 1], f16)
            nc.vector.memset(ones_t[:], 1.0)
            # dsb tables cast to f32 (is_equal scalar operand must be f32)
            dsb_t = []
            for g in range(2):
                dt_ = cpool.tile([128, int(CH[g])], f32, tag=f"dsb{g}")
                nc.vector.tensor_copy(dt_[:], dsbh_t[g][:])
                dsb_t.append(dt_)
            idh_t = cpool.tile([128, 128], f16)
            nc.vector.tensor_scalar(idh_t[:], iota_t[:], pcol_t[:],
                                    None, Alu.is_equal)
            idf_t = cpool.tile([128, 128], f32)
            nc.vector.tensor_scalar(idf_t[:], iota_t[:], pcol_t[:],
                                    None, Alu.is_equal)

            if has_bg:
                bgr_t = cpool.tile([1, 128], f32, tag="bgr")
                nc.sync.dma_start(bgr_t[:], bgr_d[:])
                # materialize bg as [128,128] via ones-outer-product
                ones_f = cpool.tile([128, 1], f32, tag="onesf")
                nc.vector.memset(ones_f[:], 1.0)
                psb = ptpool.tile([128, 128], f32, tag="pst")
                nc.tensor.matmul(psb[:], ones_f[:], bgr_t[:],
                                 start=True, stop=True)
                bg_t = cpool.tile([128, 128], f32, tag="bgt")
                nc.vector.tensor_copy(bg_t[:], psb[:])

            # ---- main: gather + one-hot segment-sum + W apply ----
            ytab = [xdq_d[0:GRP, :], xdq_d[GRP:N, :]]
            nseg = [int(np.ceil(CH[g] / SEGC)) for g in range(2)]
            seg_tiles = [[None] * nseg[g] for g in range(2)]
            oh_tiles = [[None] * nseg[g] for g in range(2)]
            seg_ptr = [0, 0]

            def ensure_seg(g, s):
                while seg_ptr[g] <= s:
                    si = seg_ptr[g]
                    ncols = min(SEGC, int(CH[g]) - si * SEGC)
                    tl = segpool.tile([128, ncols, 128], f16, tag="seg")
                    nidx = ncols * 128
                    nc.gpsimd.dma_gather(
                        tl[:], ytab[g], idx_t[g][:, si * (SEGC * 8):
                                                 si * (SEGC * 8) + ncols * 8],
                        num_idxs=nidx, num_idxs_reg=nidx,
                        elem_size=128, elem_step=128)
                    seg_tiles[g][si] = tl
                    oh = ohpool.tile([128, ncols, 128], f16, tag="oh")
                    for cc in range(ncols):
                        nc.vector.tensor_scalar(
                            oh[:, cc, :], iota_t[:],
                            dsb_t[g][:, si * SEGC + cc: si * SEGC + cc + 1],
                            None, Alu.is_equal)
                    oh_tiles[g][si] = oh
                    seg_ptr[g] += 1

            psv = pvpool.tile([1, 128], f32, tag="psv")
            first_v = [True]
            nblk = (0 if SKIP_MAIN else
                    (NB if DEBUG_BLOCKS is None else DEBUG_BLOCKS))
            for b in range(nblk):
                # residual colsum via xdT: x rows = (1/dinv) * xd rows.
                # Issued at iteration top so block b's chain overlaps its
                # own gather/matmul stream; psv group closes on final vacc.
                pstx = ptpool.tile([128, 128], f16, tag="pst")
                nc.tensor.transpose(pstx[:], xdT_t[:, b * 128:(b + 1) * 128],
                                    idh_t[:])
                xdr = hpool.tile([128, 128], f32, tag="xdr")
                nc.scalar.activation(xdr[:], pstx[:], Act.Copy)
                nc.tensor.matmul(psv[:], winv_t[:, b:b + 1], xdr[:],
                                 start=first_v[0], stop=False,
                                 skip_group_check=True)
                first_v[0] = False
                psS = pspool.tile([128, 128], f32, tag="psS")
                tot = int(ct[0][b] + ct[1][b])
                k = 0
                for g in range(2):
                    for j in range(int(ct[g][b])):
                        ci = int(off[g][b]) + j
                        s, col = divmod(ci, SEGC)
                        ensure_seg(g, s)
                        nc.tensor.matmul(psS[:], seg_tiles[g][s][:, col, :],
                                         oh_tiles[g][s][:, col, :],
                                         start=(k == 0), stop=(k == tot - 1))
                        k += 1
                # aggT: [fin, slot] PSUM -> SBUF bf16 (ACT engine)
                aggT = apool.tile([128, 128], f16, tag="agg")
                if tot == 0:
                    nc.vector.memset(aggT[:], 0.0)
                else:
                    nc.scalar.activation(aggT[:], psS[:], Act.Copy)
                # W apply + self-loop term
                psH = phpool.tile([128, 128], f32, tag="psH")
                nc.tensor.matmul(psH[:], aggT[:], wgT_t[:],
                                 start=True, stop=False)
                nc.tensor.matmul(psH[:], xdT_t[:, b * 128:(b + 1) * 128],
                                 wgT_t[:], start=False, stop=True)
                hb = hpool.tile([128, 128], f16)
                if has_bg:
                    tmp = hpool.tile([128, 128], f32, tag="tmp")
                    nc.vector.tensor_scalar(tmp[:], psH[:],
                                            dinv_t[:, b:b + 1], None, Alu.mult)
                    nc.vector.tensor_tensor(tmp[:], tmp[:], bg_t[:], Alu.add)
                    nc.scalar.activation(hb[:], tmp[:], Act.Relu)
                    if b == NB - 1:
                        nc.vector.memset(hb[NPC - (NB - 1) * 128:128, :], 0.0)
                else:
                    nc.scalar.activation(hb[:], psH[:], Act.Relu,
                                         scale=dinv_t[:, b:b + 1])
                nc.tensor.matmul(psv[:], ones_t[:], hb[:],
                                 start=False, stop=(b == nblk - 1),
                                 skip_group_check=True)

            # late-use loads (overlap the tail of the gather stream)
            w1T_t = mpool.tile([128, 512], f32, tag="w1T")
            nc.sync.dma_start(w1T_t[:], w1T_d[:])
            w2tp_t = mpool.tile([128, 1024], f32, tag="w2tp")
            nc.sync.dma_start(w2tp_t[:], w2tp_d[:])
            w3c_t = mpool.tile([128, 2], f32, tag="w3c")
            nc.sync.dma_start(w3c_t[:], w3c_d[:])
            if has_b1:
                b1c_t = mpool.tile([128, 4], f32, tag="b1c")
                nc.sync.dma_start(b1c_t[:], b1c_d[:])
            if has_b2:
                b2c_t = mpool.tile([128, 2], f32, tag="b2c")
                nc.sync.dma_start(b2c_t[:], b2c_d[:])

            # ---- v exchange: AllGather partial rows, sum on-chip ----
            vrow = mpool.tile([1, 128], f32, tag="vrow")
            if nblk == 0:
                nc.vector.memset(vrow[:], 0.0)
            else:
                nc.vector.tensor_copy(vrow[:], psv[:])
            nc.sync.dma_start(vb[:], vrow[:])
            nc.gpsimd.collective_compute(
                "AllGather", Alu.bypass, replica_groups=RG,
                ins=[vb[:]], outs=[vr[:]])
            vgt = mpool.tile([NCORES, 128], f32, tag="vgt")
            nc.sync.dma_start(vgt[:], vr[:])
            ones8 = mpool.tile([NCORES, 1], f32, tag="ones8")
            nc.vector.memset(ones8[:], 1.0)
            # ---- MLP head ----
            if SKIP_MLP:
                psv2 = pvpool.tile([1, 128], f32, tag="psv")
                nc.tensor.matmul(psv2[:], ones8[:], vgt[:],
                                 start=True, stop=True)
                vfull = mpool.tile([1, 128], f32, tag="vfull")
                nc.scalar.copy(vfull[:], psv2[:])
                nc.sync.dma_start(out_d[:], vfull[0:1, 0:1])
            else:
                psv3 = ptpool.tile([128, 1], f32, tag="pst")
                nc.tensor.matmul(psv3[:], vgt[:], ones8[:],
                                 start=True, stop=True)
                vcol = mpool.tile([128, 1], f32, tag="vcol")
                nc.vector.tensor_copy(vcol[:], psv3[:])

                a1c = []
                for m in range(4):
                    ps1 = ptpool.tile([128, 1], f32, tag="pst")
                    nc.tensor.matmul(ps1[:], w1T_t[:, m * 128:(m + 1) * 128],
                                     vcol[:], start=True, stop=True)
                    a1 = mpool.tile([128, 1], f32, tag=f"a1c{m}")
                    if has_b1:
                        nc.scalar.activation(a1[:], ps1[:], Act.Relu,
                                             bias=b1c_t[:, m:m + 1])
                    else:
                        nc.scalar.activation(a1[:], ps1[:], Act.Relu)
                    a1c.append(a1)

                a2c = []
                for m in range(2):
                    ps2 = ptpool.tile([128, 1], f32, tag="pst")
                    for kk in range(4):
                        nc.tensor.matmul(
                            ps2[:],
                            w2tp_t[:, kk * 256 + m * 128:
                                   kk * 256 + (m + 1) * 128],
                            a1c[kk][:], start=(kk == 0), stop=(kk == 3))
                    a2 = mpool.tile([128, 1], f32, tag=f"a2c{m}")
                    if has_b2:
                        nc.scalar.activation(a2[:], ps2[:], Act.Relu,
                                             bias=b2c_t[:, m:m + 1])
                    else:
                        nc.scalar.activation(a2[:], ps2[:], Act.Relu)
                    a2c.append(a2)

                ps3 = ptpool.tile([1, 1], f32, tag="ps3")
                for kk in range(2):
                    nc.tensor.matmul(ps3[:], w3c_t[:, kk:kk + 1], a2c[kk][:],
                                     start=(kk == 0), stop=(kk == 1))
                ot = mpool.tile([1, 1], f32, tag="ot")
                nc.scalar.activation(ot[:], ps3[:], Act.Copy,
                                     bias=float(b3val))
                nc.sync.dma_start(out_d[:], ot[:])

    nc.compile()
    return nc


TRACE = False
LAST_EXEC_NS = None
LAST_RESULT = None


def kernel(**inputs):
    from concourse.bass_utils import run_bass_kernel_spmd

    x = np.asarray(inputs["x"], dtype=np.float32)
    Wg = np.asarray(inputs["Wg"], dtype=np.float32)
    bg = np.asarray(inputs["bg"], dtype=np.float32)
    W1 = np.asarray(inputs["W1"], dtype=np.float32)
    b1 = np.asarray(inputs["b1"], dtype=np.float32)
    W2 = np.asarray(inputs["W2"], dtype=np.float32)
    b2 = np.asarray(inputs["b2"], dtype=np.float32)
    W3 = np.asarray(inputs["W3"], dtype=np.float32)
    b3 = np.asarray(inputs["b3"], dtype=np.float32)

    bias_info = (bool(bg.any()), bool(b1.any()), bool(b2.any()),
                 float(b3.reshape(-1)[0]))
    # LPT layout leaves empty slots scattered across blocks, which the
    # bg!=0 path cannot mask; fall back to the sequential layout there.
    dinv, in_extra, plan = _prep(inputs["edge_index"],
                                 use_lpt=not bias_info[0])
    nc = _build(plan, bias_info)

    xd = (dinv[:, None] * x).astype(BF16)           # [N, D] table
    wgT = Wg.T.astype(BF16).copy()
    w1T = W1.T.astype(np.float32).copy()            # [128, 512]
    w2tp = np.concatenate([W2.T[k * 128:(k + 1) * 128] for k in range(4)],
                          axis=1).astype(np.float32).copy()  # [128, 1024]
    w3c = W3.reshape(2, 128).T.astype(np.float32).copy()     # [128, 2]

    in_maps = []
    for c in range(NCORES):
        xdT, dvc, winvc = core_tables(c, xd, dinv, plan)
        m = {"xdq": xd, "xdT": xdT, "winvc": winvc, "dinvc": dvc,
             "wgT": wgT, "w1T": w1T, "w2tp": w2tp, "w3c": w3c,
             "idx0": in_extra[c]["idx0"], "idx1": in_extra[c]["idx1"],
             "dsb0": in_extra[c]["dsb0"], "dsb1": in_extra[c]["dsb1"]}
        if bias_info[0]:
            m["bgr"] = bg.reshape(1, 128)
        if bias_info[1]:
            m["b1c"] = b1.reshape(4, 128).T.astype(np.float32).copy()
        if bias_info[2]:
            m["b2c"] = b2.reshape(2, 128).T.astype(np.float32).copy()
        in_maps.append(m)

    res = run_bass_kernel_spmd(nc, in_maps, list(range(NCORES)), trace=TRACE)
    global LAST_EXEC_NS, LAST_RESULT
    LAST_EXEC_NS = res.exec_time_ns
    LAST_RESULT = res
    return res.results[0]["out"].reshape(1).astype(np.float32)


# revision 46
# speedup vs baseline: 1.0628x; 1.0025x over previous
"""GCN critic network kernel for 8 TRN2 NeuronCores.

Strategy (dst-shard, no message-table collective):
  The GCNConv linear commutes with the segment sum:
      out[d] = dinv_d * (sum_{s in N(d)} dinv_s * x[s]) @ Wg.T + bg
  so each core gathers pre-scaled raw rows xd = dinv*x (host-prepped bf16
  table in DRAM, a kernel input) for its own dst shard's edges and applies
  Wg once per 128-dst block after aggregation. This removes the y-table
  AllGather of the previous design entirely; the only collective left is
  the tiny [1,128] AllReduce of the pooled vector.

  - dst nodes sharded 6250/core (padded 6272 = 49 blocks of 128).
  - Edge messages: dma_gather of 256B bf16 rows (int16 indices, node table
    split in two <=32768-row groups), segment-summed per dst block via
    one-hot matmul accumulation in PSUM:  psST[fin, slot] += seg.T @ oh.
  - One-hot built per chunk with a single DVE tensor_scalar is_equal
    (iota row constant vs per-partition dst-slot scalar) - hits the
    packed-2-byte DVE fast path.
  - Self-loops folded in as one extra matmul per block from the resident
    xdT slice (no gathered self rows).
  - h = relu(dinv * (aggT.T @ WgT)); v = ones@h blocks + colsum(x_own);
    AllReduce v; tiny MLP head with host-pretransposed weights.
"""

import os
import numpy as np
import ml_dtypes

BF16 = ml_dtypes.bfloat16
N = 50000
E = 800000
D = 128
NCORES = 8
NPC = 6250          # dst nodes per core
NPAD = 6272         # padded (49 * 128)
NB = NPAD // 128    # dst blocks per core
GRP = 32768         # int16 index-group boundary (table-row space)
SEGC = int(os.environ.get("KB_SEGC", "8"))   # chunks per gather call
DDS = int(os.environ.get("KB_DDS", "65536"))
PADSLOT = 300.0     # dst-slot sentinel for padding rows (one-hot miss)

DEBUG_BLOCKS = (int(os.environ["KB_DEBUG_BLOCKS"])
                if "KB_DEBUG_BLOCKS" in os.environ else None)
SKIP_MLP = bool(os.environ.get("KB_SKIP_MLP"))
SKIP_MAIN = bool(os.environ.get("KB_SKIP_MAIN"))


def _lpt_assign(src, dst):
    """Per-core 2D-LPT node->(block, slot) map balancing per-(group, block)
    edge counts across cores; shrinks the uniform chunk-count padding."""
    c0 = np.bincount(dst[src < GRP], minlength=N)
    c1 = np.bincount(dst[src >= GRP], minlength=N)
    node_blk = np.empty(N, np.int64)
    node_slot = np.empty(N, np.int64)
    # the last-processed blocks get reduced targets so the tail drain
    # after the final gather is short (lowest-degree nodes settle there);
    # both of the last two blocks share the final gather calls
    scale = np.ones(NB)
    scale[NB - 2] = 0.6
    scale[NB - 1] = 0.25
    for c in range(NCORES):
        nodes = np.arange(c * NPC, (c + 1) * NPC)
        a0, a1 = c0[nodes].astype(np.float64), c1[nodes].astype(np.float64)
        order = np.argsort(-(a0 + a1), kind="stable")
        T0 = np.maximum(a0.sum() * scale / scale.sum(), 1.0)
        T1 = np.maximum(a1.sum() * scale / scale.sum(), 1.0)
        w0 = np.zeros(NB)
        w1 = np.zeros(NB)
        ns = np.zeros(NB, np.int64)
        for j in order:
            pen = np.where(ns < 128,
                           np.maximum((w0 + a0[j]) / T0, (w1 + a1[j]) / T1),
                           np.inf)
            b = int(np.argmin(pen))
            node_blk[nodes[j]] = b
            node_slot[nodes[j]] = ns[b]
            w0[b] += a0[j]
            w1[b] += a1[j]
            ns[b] += 1
    return node_blk, node_slot


def _prep(edge_index, use_lpt=True):
    """Host-side graph prep: per-core chunked edge layout + uniform plan."""
    src = np.asarray(edge_index[0]).astype(np.int64)
    dst = np.asarray(edge_index[1]).astype(np.int64)

    deg = np.bincount(dst, minlength=N).astype(np.float64) + 1.0
    dinv = (1.0 / np.sqrt(deg)).astype(np.float32)

    if use_lpt and not os.environ.get("KB_NOLPT"):
        node_blk, node_slot = _lpt_assign(src, dst)
    else:
        local = np.arange(N) % NPC
        node_blk = local >> 7
        node_slot = local & 127

    per_core = []
    cnt = np.zeros((NCORES, 2, NB), dtype=np.int64)
    for c in range(NCORES):
        lo, hi = c * NPC, (c + 1) * NPC
        m = (dst >= lo) & (dst < hi)
        es = src[m]
        ds = dst[m]
        g = (es >= GRP).astype(np.int64)
        blk = node_blk[ds]
        slot = node_slot[ds]
        # sort by (group, block, src) - src order improves HBM locality
        order = np.lexsort((es, blk, g))
        es, slot, g, blk = es[order], slot[order], g[order], blk[order]
        np.add.at(cnt[c], (g, blk), 1)
        per_core.append((es, slot, g, blk))

    # uniform chunk counts per (group, block) across cores (SPMD program)
    ct = np.ceil(cnt.max(axis=0) / 128.0).astype(np.int64)  # [2, NB]
    CH = ct.sum(axis=1)                                     # chunks per group
    off = np.zeros((2, NB), dtype=np.int64)
    off[:, 1:] = np.cumsum(ct, axis=1)[:, :-1]

    def wrap_idx(loc):
        nch = len(loc) // 128
        cols = []
        for s0 in range(0, nch, SEGC):
            seg = loc[s0 * 128: min(nch, s0 + SEGC) * 128]
            a = seg.reshape(-1, 16).T.astype(np.int16)        # [16, cols_s]
            cols.append(np.tile(a, (8, 1)))                   # [128, cols_s]
        return np.concatenate(cols, axis=1)

    in_extra = []
    for c in range(NCORES):
        es, slot, gs, blks = per_core[c]
        d = {}
        for g in range(2):
            loc = np.zeros(CH[g] * 128, dtype=np.int64)
            slo = np.full(CH[g] * 128, PADSLOT, dtype=np.float64)
            sel = gs == g
            ep, sl, bl = es[sel] - g * GRP, slot[sel], blks[sel]
            bstart = np.zeros(NB, dtype=np.int64)
            bstart[1:] = np.cumsum(np.bincount(bl, minlength=NB))[:-1]
            pos = off[g][bl] * 128 + (np.arange(len(ep)) - bstart[bl])
            loc[pos] = ep
            slo[pos] = sl
            d[f"idx{g}"] = wrap_idx(loc)
            d[f"dsb{g}"] = slo.reshape(-1, 128).T.astype(BF16)
        in_extra.append(d)

    plan = {"ct": ct, "CH": CH, "off": off,
            "nblk": node_blk, "nslot": node_slot}
    return dinv, in_extra, plan


def core_tables(c, xd, dinv, plan):
    """Per-core dst-side tables in the (possibly LPT-permuted) layout."""
    nblk, nslot = plan["nblk"], plan["nslot"]
    lo, hi = c * NPC, (c + 1) * NPC
    pos = nblk[lo:hi] * 128 + nslot[lo:hi]
    xdT = np.zeros((128, NPAD), dtype=BF16)
    xdT[:, pos] = xd[lo:hi].T
    dv = np.zeros(NPAD, dtype=np.float32)
    dv[pos] = dinv[lo:hi]
    dvc = dv.reshape(NB, 128).T.copy()
    wv = np.zeros(NPAD, dtype=np.float32)
    wv[pos] = 1.0 / dinv[lo:hi]
    winvc = wv.reshape(NB, 128).T.copy()
    return xdT, dvc, winvc


def _build(plan, bias_info):
    import concourse.bacc as bacc
    import concourse.tile as tile
    from concourse import mybir

    f32 = mybir.dt.float32
    f16 = mybir.dt.bfloat16
    i16 = mybir.dt.int16
    Alu = mybir.AluOpType
    Act = mybir.ActivationFunctionType

    ct, CH, off = plan["ct"], plan["CH"], plan["off"]
    has_bg, has_b1, has_b2, b3val = bias_info

    nc = bacc.Bacc("TRN2", target_bir_lowering=False, debug=False,
                   num_devices=NCORES,
                   dynamic_dma_scratch_size=DDS)

    def din(name, shape, dt=f32):
        return nc.dram_tensor(name, list(shape), dt, kind="ExternalInput")

    xdq_d = din("xdq", [N, D], f16)          # dinv*x, full table (bf16)
    xdT_d = din("xdT", [128, NPAD], f16)     # (dinv*x)[own].T
    dinvc_d = din("dinvc", [128, NB])
    winv_d = din("winvc", [128, NB])         # 1/dinv per (slot, block)
    wgT_d = din("wgT", [D, D], f16)          # Wg.T
    w1T_d = din("w1T", [128, 512])           # W1.T
    w2tp_d = din("w2tp", [128, 1024])        # W2.T row-blocks packed
    w3c_d = din("w3c", [128, 2])             # W3.T column chunks
    idx_d = [din(f"idx{g}", [128, int(CH[g]) * 8], i16) for g in range(2)]
    dsb_d = [din(f"dsb{g}", [128, int(CH[g])], f16) for g in range(2)]
    bgr_d = din("bgr", [1, 128]) if has_bg else None
    b1c_d = din("b1c", [128, 4]) if has_b1 else None
    b2c_d = din("b2c", [128, 2]) if has_b2 else None
    out_d = nc.dram_tensor("out", [1, 1], f32, kind="ExternalOutput")

    vb = nc.dram_tensor("vb", [1, 128], f32)
    vr = nc.dram_tensor("vr", [NCORES, 128], f32, addr_space="Shared")

    RG = [list(range(NCORES))]

    with tile.TileContext(nc) as tc:
        with (
            tc.tile_pool(name="const", bufs=1) as cpool,
            tc.tile_pool(name="seg", bufs=12) as segpool,
            tc.tile_pool(name="oh", bufs=12) as ohpool,
            tc.tile_pool(name="agg", bufs=3) as apool,
            tc.tile_pool(name="hb", bufs=3) as hpool,
            tc.tile_pool(name="mlp", bufs=1) as mpool,
            tc.tile_pool(name="psv", bufs=1, space="PSUM") as pvpool,
            tc.tile_pool(name="psS", bufs=3, space="PSUM") as pspool,
            tc.tile_pool(name="psH", bufs=2, space="PSUM") as phpool,
            tc.tile_pool(name="psT", bufs=1, space="PSUM") as ptpool,
        ):
            # ---- prefetch: gather-critical tables first ----
            # idx0 split so the first gather segment unblocks early
            idx_t = []
            it0 = cpool.tile([128, int(CH[0]) * 8], i16, tag="idx0")
            c_split = min(4 * SEGC * 8, int(CH[0]) * 8)
            nc.sync.dma_start(it0[:, 0:c_split], idx_d[0][:, 0:c_split])
            dsbh_t = []
            dh0 = cpool.tile([128, int(CH[0])], f16, tag="dsbh0")
            nc.sync.dma_start(dh0[:], dsb_d[0][:])
            nc.sync.dma_start(it0[:, c_split:int(CH[0]) * 8],
                              idx_d[0][:, c_split:int(CH[0]) * 8])
            idx_t.append(it0)
            dsbh_t.append(dh0)
            it1 = cpool.tile([128, int(CH[1]) * 8], i16, tag="idx1")
            nc.sync.dma_start(it1[:], idx_d[1][:])
            idx_t.append(it1)
            dh1 = cpool.tile([128, int(CH[1])], f16, tag="dsbh1")
            nc.sync.dma_start(dh1[:], dsb_d[1][:])
            dsbh_t.append(dh1)
            dinv_t = cpool.tile([128, NB], f32)
            nc.sync.dma_start(dinv_t[:], dinvc_d[:])
            winv_t = cpool.tile([128, NB], f32, tag="winv")
            nc.sync.dma_start(winv_t[:], winv_d[:])
            wgT_t = cpool.tile([128, 128], f16, tag="wgT")
            nc.sync.dma_start(wgT_t[:], wgT_d[:])
            xdT_t = cpool.tile([128, NPAD], f16, tag="xdT")
            nc.sync.dma_start(xdT_t[:], xdT_d[:])

            # on-chip constants (no DMA): iota row, identities, ones
            iota_t = cpool.tile([128, 128], f16)
            nc.gpsimd.iota(iota_t[:], [[1, 128]], channel_multiplier=0,
                           allow_small_or_imprecise_dtypes=True)
            pcol_t = cpool.tile([128, 1], f32, tag="pcol")
            nc.gpsimd.iota(pcol_t[:], [[0, 1]], channel_multiplier=1,
                           allow_small_or_imprecise_dtypes=True)
            ones_t = cpool.tile([128, 1], f16)
            nc.vector.memset(ones_t[:], 1.0)
            # dsb tables cast to f32 (is_equal scalar operand must be f32)
            dsb_t = []
            for g in range(2):
                dt_ = cpool.tile([128, int(CH[g])], f32, tag=f"dsb{g}")
                nc.vector.tensor_copy(dt_[:], dsbh_t[g][:])
                dsb_t.append(dt_)
            idh_t = cpool.tile([128, 128], f16)
            nc.vector.tensor_scalar(idh_t[:], iota_t[:], pcol_t[:],
                                    None, Alu.is_equal)
            idf_t = cpool.tile([128, 128], f32)
            nc.vector.tensor_scalar(idf_t[:], iota_t[:], pcol_t[:],
                                    None, Alu.is_equal)

            if has_bg:
                bgr_t = cpool.tile([1, 128], f32, tag="bgr")
                nc.sync.dma_start(bgr_t[:], bgr_d[:])
                # materialize bg as [128,128] via ones-outer-product
                ones_f = cpool.tile([128, 1], f32, tag="onesf")
                nc.vector.memset(ones_f[:], 1.0)
                psb = ptpool.tile([128, 128], f32, tag="pst")
                nc.tensor.matmul(psb[:], ones_f[:], bgr_t[:],
                                 start=True, stop=True)
                bg_t = cpool.tile([128, 128], f32, tag="bgt")
                nc.vector.tensor_copy(bg_t[:], psb[:])

            # ---- main: gather + one-hot segment-sum + W apply ----
            ytab = [xdq_d[0:GRP, :], xdq_d[GRP:N, :]]
            nseg = [int(np.ceil(CH[g] / SEGC)) for g in range(2)]
            seg_tiles = [[None] * nseg[g] for g in range(2)]
            oh_tiles = [[None] * nseg[g] for g in range(2)]
            seg_ptr = [0, 0]

            def ensure_seg(g, s):
                while seg_ptr[g] <= s:
                    si = seg_ptr[g]
                    ncols = min(SEGC, int(CH[g]) - si * SEGC)
                    tl = segpool.tile([128, ncols, 128], f16, tag="seg")
                    nidx = ncols * 128
                    nc.gpsimd.dma_gather(
                        tl[:], ytab[g], idx_t[g][:, si * (SEGC * 8):
                                                 si * (SEGC * 8) + ncols * 8],
                        num_idxs=nidx, num_idxs_reg=nidx,
                        elem_size=128, elem_step=128)
                    seg_tiles[g][si] = tl
                    oh = ohpool.tile([128, ncols, 128], f16, tag="oh")
                    for cc in range(ncols):
                        nc.vector.tensor_scalar(
                            oh[:, cc, :], iota_t[:],
                            dsb_t[g][:, si * SEGC + cc: si * SEGC + cc + 1],
                            None, Alu.is_equal)
                    oh_tiles[g][si] = oh
                    seg_ptr[g] += 1

            psv = pvpool.tile([1, 128], f32, tag="psv")
            first_v = [True]
            nblk = (0 if SKIP_MAIN else
                    (NB if DEBUG_BLOCKS is None else DEBUG_BLOCKS))
            for b in range(nblk):
                # residual colsum via xdT: x rows = (1/dinv) * xd rows.
                # Issued at iteration top so block b's chain overlaps its
                # own gather/matmul stream; psv group closes on final vacc.
                pstx = ptpool.tile([128, 128], f16, tag="pst")
                nc.tensor.transpose(pstx[:], xdT_t[:, b * 128:(b + 1) * 128],
                                    idh_t[:])
                xdr = hpool.tile([128, 128], f32, tag="xdr")
                nc.scalar.activation(xdr[:], pstx[:], Act.Copy)
                nc.tensor.matmul(psv[:], winv_t[:, b:b + 1], xdr[:],
                                 start=first_v[0], stop=False,
                                 skip_group_check=True)
                first_v[0] = False
                psS = pspool.tile([128, 128], f32, tag="psS")
                tot = int(ct[0][b] + ct[1][b])
                k = 0
                for g in range(2):
                    for j in range(int(ct[g][b])):
                        ci = int(off[g][b]) + j
                        s, col = divmod(ci, SEGC)
                        ensure_seg(g, s)
                        nc.tensor.matmul(psS[:], seg_tiles[g][s][:, col, :],
                                         oh_tiles[g][s][:, col, :],
                                         start=(k == 0), stop=(k == tot - 1))
                        k += 1
                # aggT: [fin, slot] PSUM -> SBUF bf16 (ACT engine)
                aggT = apool.tile([128, 128], f16, tag="agg")
                if tot == 0:
                    nc.vector.memset(aggT[:], 0.0)
                else:
                    nc.scalar.activation(aggT[:], psS[:], Act.Copy)
                # W apply + self-loop term
                psH = phpool.tile([128, 128], f32, tag="psH")
                nc.tensor.matmul(psH[:], aggT[:], wgT_t[:],
                                 start=True, stop=False)
                nc.tensor.matmul(psH[:], xdT_t[:, b * 128:(b + 1) * 128],
                                 wgT_t[:], start=False, stop=True)
                hb = hpool.tile([128, 128], f16)
                if has_bg:
                    tmp = hpool.tile([128, 128], f32, tag="tmp")
                    nc.vector.tensor_scalar(tmp[:], psH[:],
                                            dinv_t[:, b:b + 1], None, Alu.mult)
                    nc.vector.tensor_tensor(tmp[:], tmp[:], bg_t[:], Alu.add)
                    nc.scalar.activation(hb[:], tmp[:], Act.Relu)
                    if b == NB - 1:
                        nc.vector.memset(hb[NPC - (NB - 1) * 128:128, :], 0.0)
                else:
                    nc.scalar.activation(hb[:], psH[:], Act.Relu,
                                         scale=dinv_t[:, b:b + 1])
                nc.tensor.matmul(psv[:], ones_t[:], hb[:],
                                 start=False, stop=(b == nblk - 1),
                                 skip_group_check=True)

            # late-use loads (overlap the tail of the gather stream)
            w1T_t = mpool.tile([128, 512], f32, tag="w1T")
            nc.sync.dma_start(w1T_t[:], w1T_d[:])
            w2tp_t = mpool.tile([128, 1024], f32, tag="w2tp")
            nc.sync.dma_start(w2tp_t[:], w2tp_d[:])
            w3c_t = mpool.tile([128, 2], f32, tag="w3c")
            nc.sync.dma_start(w3c_t[:], w3c_d[:])
            if has_b1:
                b1c_t = mpool.tile([128, 4], f32, tag="b1c")
                nc.sync.dma_start(b1c_t[:], b1c_d[:])
            if has_b2:
                b2c_t = mpool.tile([128, 2], f32, tag="b2c")
                nc.sync.dma_start(b2c_t[:], b2c_d[:])

            # ---- v exchange: AllGather partial rows, sum on-chip ----
            vrow = mpool.tile([1, 128], f32, tag="vrow")
            if nblk == 0:
                nc.vector.memset(vrow[:], 0.0)
            else:
                nc.vector.tensor_copy(vrow[:], psv[:])
            nc.sync.dma_start(vb[:], vrow[:])
            nc.gpsimd.collective_compute(
                "AllGather", Alu.bypass, replica_groups=RG,
                ins=[vb[:]], outs=[vr[:]])
            vgt = mpool.tile([NCORES, 128], f32, tag="vgt")
            nc.sync.dma_start(vgt[:], vr[:])
            ones8 = mpool.tile([NCORES, 1], f32, tag="ones8")
            nc.vector.memset(ones8[:], 1.0)
            # ---- MLP head ----
            if SKIP_MLP:
                psv2 = pvpool.tile([1, 128], f32, tag="psv")
                nc.tensor.matmul(psv2[:], ones8[:], vgt[:],
                                 start=True, stop=True)
                vfull = mpool.tile([1, 128], f32, tag="vfull")
                nc.scalar.copy(vfull[:], psv2[:])
                nc.sync.dma_start(out_d[:], vfull[0:1, 0:1])
            else:
                psv3 = ptpool.tile([128, 1], f32, tag="pst")
                nc.tensor.matmul(psv3[:], vgt[:], ones8[:],
                                 start=True, stop=True)
                vcol = mpool.tile([128, 1], f32, tag="vcol")
                nc.vector.tensor_copy(vcol[:], psv3[:])

                a1c = []
                for m in range(4):
                    ps1 = ptpool.tile([128, 1], f32, tag="pst")
                    nc.tensor.matmul(ps1[:], w1T_t[:, m * 128:(m + 1) * 128],
                                     vcol[:], start=True, stop=True)
                    a1 = mpool.tile([128, 1], f32, tag=f"a1c{m}")
                    if has_b1:
                        nc.scalar.activation(a1[:], ps1[:], Act.Relu,
                                             bias=b1c_t[:, m:m + 1])
                    else:
                        nc.scalar.activation(a1[:], ps1[:], Act.Relu)
                    a1c.append(a1)

                a2c = []
                for m in range(2):
                    ps2 = ptpool.tile([128, 1], f32, tag="pst")
                    for kk in range(4):
                        nc.tensor.matmul(
                            ps2[:],
                            w2tp_t[:, kk * 256 + m * 128:
                                   kk * 256 + (m + 1) * 128],
                            a1c[kk][:], start=(kk == 0), stop=(kk == 3))
                    a2 = mpool.tile([128, 1], f32, tag=f"a2c{m}")
                    if has_b2:
                        nc.scalar.activation(a2[:], ps2[:], Act.Relu,
                                             bias=b2c_t[:, m:m + 1])
                    else:
                        nc.scalar.activation(a2[:], ps2[:], Act.Relu)
                    a2c.append(a2)

                ps3 = ptpool.tile([1, 1], f32, tag="ps3")
                for kk in range(2):
                    nc.tensor.matmul(ps3[:], w3c_t[:, kk:kk + 1], a2c[kk][:],
                                     start=(kk == 0), stop=(kk == 1))
                ot = mpool.tile([1, 1], f32, tag="ot")
                nc.scalar.activation(ot[:], ps3[:], Act.Copy,
                                     bias=float(b3val))
                nc.sync.dma_start(out_d[:], ot[:])

    nc.compile()
    return nc


TRACE = False
LAST_EXEC_NS = None
LAST_RESULT = None


def kernel(**inputs):
    from concourse.bass_utils import run_bass_kernel_spmd

    x = np.asarray(inputs["x"], dtype=np.float32)
    Wg = np.asarray(inputs["Wg"], dtype=np.float32)
    bg = np.asarray(inputs["bg"], dtype=np.float32)
    W1 = np.asarray(inputs["W1"], dtype=np.float32)
    b1 = np.asarray(inputs["b1"], dtype=np.float32)
    W2 = np.asarray(inputs["W2"], dtype=np.float32)
    b2 = np.asarray(inputs["b2"], dtype=np.float32)
    W3 = np.asarray(inputs["W3"], dtype=np.float32)
    b3 = np.asarray(inputs["b3"], dtype=np.float32)

    bias_info = (bool(bg.any()), bool(b1.any()), bool(b2.any()),
                 float(b3.reshape(-1)[0]))
    # LPT layout leaves empty slots scattered across blocks, which the
    # bg!=0 path cannot mask; fall back to the sequential layout there.
    dinv, in_extra, plan = _prep(inputs["edge_index"],
                                 use_lpt=not bias_info[0])
    nc = _build(plan, bias_info)

    xd = (dinv[:, None] * x).astype(BF16)           # [N, D] table
    wgT = Wg.T.astype(BF16).copy()
    w1T = W1.T.astype(np.float32).copy()            # [128, 512]
    w2tp = np.concatenate([W2.T[k * 128:(k + 1) * 128] for k in range(4)],
                          axis=1).astype(np.float32).copy()  # [128, 1024]
    w3c = W3.reshape(2, 128).T.astype(np.float32).copy()     # [128, 2]

    in_maps = []
    for c in range(NCORES):
        xdT, dvc, winvc = core_tables(c, xd, dinv, plan)
        m = {"xdq": xd, "xdT": xdT, "winvc": winvc, "dinvc": dvc,
             "wgT": wgT, "w1T": w1T, "w2tp": w2tp, "w3c": w3c,
             "idx0": in_extra[c]["idx0"], "idx1": in_extra[c]["idx1"],
             "dsb0": in_extra[c]["dsb0"], "dsb1": in_extra[c]["dsb1"]}
        if bias_info[0]:
            m["bgr"] = bg.reshape(1, 128)
        if bias_info[1]:
            m["b1c"] = b1.reshape(4, 128).T.astype(np.float32).copy()
        if bias_info[2]:
            m["b2c"] = b2.reshape(2, 128).T.astype(np.float32).copy()
        in_maps.append(m)

    res = run_bass_kernel_spmd(nc, in_maps, list(range(NCORES)), trace=TRACE)
    global LAST_EXEC_NS, LAST_RESULT
    LAST_EXEC_NS = res.exec_time_ns
    LAST_RESULT = res
    return res.results[0]["out"].reshape(1).astype(np.float32)
